# revision 1
# baseline (speedup 1.0000x reference)
"""MFGCGRU (graph-conv GRU cell) Trainium2 kernel.

Strategy: data-parallel over batch B=32 across 8 NeuronCores (4 batches
per core). All NxN supports replicated per core. Host pre-transposes
everything so the device never transposes:

  - adjacency matrices are passed as S^T [m, n] (bf16) and used as the
    *moving* matmul operand,
  - activations live feature-major: x_cat^T [66, N] with rows 0:64 = h,
    rows 64:66 = inputs (kernel rows permuted to match),
  - the diffusion conv is computed kernel-first:
        S_m @ (X @ k_m)  ==  (S_m X) k_m
    with Y_m = X @ k_m packed [node, 128] = [k_r | k_u] for the r/u pass
    (and batch-pairs for the c pass) so the PE always runs 128 wide,
  - the attention support is built unnormalized as e^T = exp(K Q^T / 8);
    its row-normalizer 1/d is applied to the e-contribution after the
    node contraction via a second PSUM accumulator.
"""

import contextlib
import os

import numpy as np
import ml_dtypes

import concourse.bass as bass
import concourse.bacc as bacc
import concourse.tile as tile
from concourse import mybir
from concourse.bass_utils import run_bass_kernel_spmd

F32 = mybir.dt.float32
BF16 = mybir.dt.bfloat16
AF = mybir.ActivationFunctionType

B, N, DIN, U, FD, SD = 32, 2048, 2, 64, 32, 64
NCORES = 8
BL = B // NCORES          # batches per core
NTW = 512                 # n-tile width
NT = N // NTW             # 4 n-tiles
NBW = 128                 # node-block width
NB = N // NBW             # 16 node blocks
FROWS = DIN + U           # 66


def _build_program():
    nc = bacc.Bacc("TRN2", debug=False, num_devices=NCORES)

    d = {}

    def din(name, shape, dt):
        d[name] = nc.dram_tensor(name, shape, dt, kind="ExternalInput").ap()

    din("xT", [BL, FROWS, N], BF16)
    din("hT", [BL, U, N], F32)
    din("a1T", [N, N], BF16)
    din("a2T", [N, N], BF16)
    din("fsT", [FD + SD, N], F32)
    din("wq", [FD, U], F32)
    din("wk", [FD, U], F32)
    din("ws1", [FD + SD, U], F32)
    din("bs1v", [U, 1], F32)
    din("ws2", [U, 1], F32)
    din("bs2v", [1, 1], F32)
    din("kkall", [FROWS, 3 * 2 * U], BF16)
    din("kk0", [FROWS, 2 * U], BF16)
    din("kcall", [FROWS, 3 * U], BF16)
    din("kc0", [FROWS, U], BF16)
    din("bru", [2 * U, 1], F32)
    din("bc2", [2 * U, 1], F32)
    out_h = nc.dram_tensor("out", [BL, U, N], F32, kind="ExternalOutput").ap()
    uscr = nc.dram_tensor("uscr", [BL, U, N], F32).ap()

    with tile.TileContext(nc) as tc:
        _emit(tc, d, out_h, uscr)
    nc.compile()
    return nc


def _emit(tc, d, out_h, uscr):
    nc = tc.nc
    ctx = contextlib.ExitStack()
    const = ctx.enter_context(tc.tile_pool(name="const", bufs=1))
    persist = ctx.enter_context(tc.tile_pool(name="persist", bufs=1))
    adjp = ctx.enter_context(tc.tile_pool(name="adjp", bufs=4))
    etp = ctx.enter_context(tc.tile_pool(name="etp", bufs=1))
    ypool = ctx.enter_context(tc.tile_pool(name="ypool", bufs=1))
    stage = ctx.enter_context(tc.tile_pool(name="stage", bufs=2))
    p3p = ctx.enter_context(tc.tile_pool(name="p3p", bufs=2))
    psacc = ctx.enter_context(tc.tile_pool(name="psacc", bufs=5, space="PSUM"))
    psscr = ctx.enter_context(tc.tile_pool(name="psscr", bufs=3, space="PSUM"))

    # ---- constants / weights in SBUF ----
    def cload(name, shape=None, dt=None):
        ap = d[name]
        t = const.tile(list(ap.shape) if shape is None else shape,
                       ap.dtype if dt is None else dt, name=f"c_{name}")
        nc.sync.dma_start(out=t, in_=ap)
        return t

    fsT = const.tile([FD + SD, N], F32, name="c_fsT")
    nc.sync.dma_start(out=fsT[0:FD, :], in_=d["fsT"][0:FD, :])
    wq = cload("wq")
    wk = cload("wk")
    nc.sync.dma_start(out=fsT[FD:, :], in_=d["fsT"][FD:, :])
    ws1 = cload("ws1")
    bs1v = cload("bs1v")
    ws2 = cload("ws2")
    bs2v = cload("bs2v")

    kkall = cload("kkall")
    kcall = cload("kcall")
    kk0 = cload("kk0")
    kc0 = cload("kc0")
    bru = cload("bru")
    bc2 = cload("bc2")

    # ---- persistent activations ----
    xT = [persist.tile([FROWS, N], BF16, name=f"xT{b}", tag=f"xT{b}")
          for b in range(BL)]
    for b in range(BL):
        nc.sync.dma_start(out=xT[b], in_=d["xT"][b])

    ones_col = const.tile([128, 1], BF16, name="ones_col")
    nc.vector.memset(ones_col, 1.0)
    ones_row = const.tile([1, 128], F32, name="ones_row")
    nc.vector.memset(ones_row, 1.0)

    QT = persist.tile([U, N], BF16, name="QT", tag="QT")
    KT = persist.tile([U, N], BF16, name="KT", tag="KT")
    s_row = persist.tile([1, N], F32, name="s_row", tag="s_row")
    rdbc = [persist.tile([128, NTW], BF16, name=f"rdbc{t}", tag=f"rdbc{t}")
            for t in range(NT)]

    # ---- prelude: Q^T, K^T, s ----
    for t in range(NT):
        sl = slice(t * NTW, (t + 1) * NTW)
        pq = psscr.tile([U, NTW], F32, name="pq", tag="scr")
        nc.tensor.matmul(pq, wq, fsT[0:FD, sl], start=True, stop=True)
        nc.scalar.activation(QT[:, sl], pq, AF.Relu)
        pk = psscr.tile([U, NTW], F32, name="pk", tag="scr")
        nc.tensor.matmul(pk, wk, fsT[0:FD, sl], start=True, stop=True)
        nc.scalar.activation(KT[:, sl], pk, AF.Relu)
        ps1 = psscr.tile([U, NTW], F32, name="ps1", tag="scr")
        nc.tensor.matmul(ps1, ws1, fsT[:, sl], start=True, stop=True)
        s1t = stage.tile([U, NTW], F32, name="s1t", tag="s1t")
        nc.scalar.activation(s1t, ps1, AF.Relu, bias=bs1v)
        ps2 = psscr.tile([1, NTW], F32, name="ps2", tag="scr")
        nc.tensor.matmul(ps2, ws2, s1t, start=True, stop=True)
        nc.scalar.activation(s_row[:, sl], ps2, AF.Relu, bias=bs2v)

    # ---- phase-1 Y tiles: Y[m,b] = X_b @ [k_r[m]|k_u[m]], all m in one
    # MM: stored as [128, NB, 3, 128]: [node%128, node//128, m, u']
    y = [ypool.tile([NBW, NB, 3, 2 * U], BF16, name=f"y_{b}", tag=f"y{b}")
         for b in range(BL)]
    for b in range(BL):
        for j in range(NB):
            nsl = slice(j * NBW, (j + 1) * NBW)
            py = psscr.tile([NBW, 3 * 2 * U], F32, name="py", tag="scr")
            nc.tensor.matmul(py, xT[b][:, nsl], kkall, start=True, stop=True)
            nc.vector.tensor_copy(
                y[b][:, j, :, :],
                py.rearrange("p (m u) -> p m u", m=3))

    def e_thunks(t, et):
        """Thunks generating e^T[:, t] = exp(K Q^T / 8) into et, one
        node-block per call — interleaved into adjacency groups so the
        ACT exp evacuations hide under PE matmul streaming."""
        sl = slice(t * NTW, (t + 1) * NTW)

        def mk(j):
            def f():
                pe = psscr.tile([NBW, NTW], F32, name="pe", tag="scr")
                nc.tensor.matmul(pe, KT[:, j * NBW:(j + 1) * NBW], QT[:, sl],
                                 start=True, stop=True)
                nc.scalar.activation(et[:, j, :], pe, AF.Exp, scale=0.125)
            return f
        return [mk(j) for j in range(NB)]

    def d_thunks(et, pd):
        def mk(j):
            def f():
                nc.tensor.matmul(pd, ones_col, et[:, j, :],
                                 start=(j == 0), stop=(j == NB - 1))
            return f
        return [mk(j) for j in range(NB)]

    def interleave(main, extra, ratio=2):
        """Emit `ratio` thunks from main per one from extra."""
        mi = ei = 0
        while mi < len(main) or ei < len(extra):
            for _ in range(ratio):
                if mi < len(main):
                    main[mi](); mi += 1
            if ei < len(extra):
                extra[ei](); ei += 1

    def adjslice(name, t):
        sl = d[name][:, t * NTW:(t + 1) * NTW]
        a = adjp.tile([NBW, NB, NTW], BF16, name=f"sl_{name}", tag="adj")
        nc.sync.dma_start(out=a, in_=sl.rearrange("(j p) w -> p j w", p=NBW))
        return a

    # =================== phase 1: r & u gates ===================
    def a_thunks1(b, sl, a1, a2, pa):
        th = [lambda: nc.tensor.matmul(pa, kk0, xT[b][:, sl],
                                       start=True, stop=False)]
        for m, asl in ((0, a1), (1, a2)):
            for j in range(NB):
                def f(m=m, asl=asl, j=j):
                    nc.tensor.matmul(pa, y[b][:, j, m, :], asl[:, j, :],
                                     start=False,
                                     stop=(m == 1 and j == NB - 1))
                th.append(f)
        return th

    def agroup1(b, sl, a1, a2, extra=(), ratio=2):
        pa = psacc.tile([128, NTW], F32, name="pa", tag="acc")
        interleave(a_thunks1(b, sl, a1, a2, pa), list(extra), ratio=ratio)
        return pa

    def bgroup1(b, t, sl, et, pa):
        pb = psacc.tile([128, NTW], F32, name="pb", tag="acc")
        for j in range(NB):
            nc.tensor.matmul(pb, y[b][:, j, 2, :], et[:, j, :],
                             start=(j == 0), stop=(j == NB - 1))
        tmp = stage.tile([128, NTW], F32, name="tmp", tag="tmp")
        nc.vector.tensor_mul(tmp, pb, rdbc[t])
        ssum = stage.tile([128, NTW], F32, name="ssum", tag="ssum")
        nc.vector.tensor_add(ssum, pa, tmp)
        sig = stage.tile([128, NTW], F32, name="sig", tag="sig")
        nc.scalar.activation(sig, ssum, AF.Sigmoid, scale=0.25, bias=bru)
        # rh -> x_cat_c rows 0:64 in place; u -> DRAM scratch
        nc.vector.tensor_mul(xT[b][0:U, sl], sig[0:U, :], xT[b][0:U, sl])
        nc.sync.dma_start(out=uscr[b][:, sl], in_=sig[U:128, :])

    preload = (adjslice("a1T", 0), adjslice("a2T", 0))

    def ycgen_thunks(yct, p):
        th = []
        for half in range(2):
            b = 2 * p + half
            usl = slice(half * U, (half + 1) * U)
            for j in range(NB):
                def f(b=b, usl=usl, j=j, yct=yct):
                    nsl = slice(j * NBW, (j + 1) * NBW)
                    pyc = psscr.tile([NBW, 3 * U], F32, name="pyc", tag="scr")
                    nc.tensor.matmul(pyc, xT[b][:, nsl], kcall,
                                     start=True, stop=True)
                    nc.vector.tensor_copy(
                        yct[:, j, :, usl],
                        pyc.rearrange("p (m u) -> p m u", m=3))
                th.append(f)
        return th

    yc = [None, None]

    for t in range(NT):
        sl = slice(t * NTW, (t + 1) * NTW)
        if t == 0:
            a1, a2 = preload
        else:
            a1 = adjslice("a1T", t)
            a2 = adjslice("a2T", t)
        et = etp.tile([NBW, NB, NTW], BF16, name="et", tag="et")
        pd = psscr.tile([1, NTW], F32, name="pd", tag="scr")
        pa0 = agroup1(0, sl, a1, a2)
        pa1 = agroup1(1, sl, a1, a2, e_thunks(t, et))
        pa2 = agroup1(2, sl, a1, a2, d_thunks(et, pd))

        # d[n] = s[n] + colsum(e^T)[n]; rdbc[t][p, n] = 1 / d[n]
        dsb = stage.tile([1, NTW], F32, name="dsb", tag="dsb")
        nc.vector.tensor_add(dsb, pd, s_row[:, sl])
        rds = stage.tile([1, NTW], F32, name="rds", tag="dsb")
        nc.vector.reciprocal(rds, dsb)
        pr = psscr.tile([128, NTW], F32, name="pr", tag="scr")
        nc.tensor.matmul(pr, ones_row, rds, start=True, stop=True)
        nc.scalar.activation(rdbc[t], pr, AF.Copy)

        bgroup1(0, t, sl, et, pa0)
        bgroup1(1, t, sl, et, pa1)
        if t == NT - 1:
            yc[0] = ypool.tile([NBW, NB, 3, 2 * U], BF16, name="yc_0",
                               tag="y0")
            pa3 = agroup1(3, sl, a1, a2, ycgen_thunks(yc[0], 0), ratio=1)
        else:
            pa3 = agroup1(3, sl, a1, a2)
        bgroup1(2, t, sl, et, pa2)
        bgroup1(3, t, sl, et, pa3)

    # =================== phase 2+3: c gate & h_new ===================
    # Yc[pair] = [Xc_b0 @ kc[m] | Xc_b1 @ kc[m]] packed [128, NB, 3, 128]
    # (yc[0] was already generated inside phase-1's final A-group)
    yc[1] = ypool.tile([NBW, NB, 3, 2 * U], BF16, name="yc_1", tag="y1")
    for f in ycgen_thunks(yc[1], 1):
        f()

    for t in range(NT):
        sl = slice(t * NTW, (t + 1) * NTW)
        a1 = adjslice("a1T", t)
        a2 = adjslice("a2T", t)
        et = etp.tile([NBW, NB, NTW], BF16, name="et2", tag="et")
        pas = []
        for p in range(BL // 2):
            b0, b1 = 2 * p, 2 * p + 1
            pa = psacc.tile([128, NTW], F32, name="pa2", tag="acc")
            th = [lambda pa=pa, p=p: nc.tensor.matmul(
                      pa, yc[p][:, 0, 0, :], a1[:, 0, :], start=True, stop=False),
                  lambda pa=pa, b0=b0: nc.tensor.matmul(
                      pa[0:U, :], kc0, xT[b0][:, sl], start=False, stop=False),
                  lambda pa=pa, b1=b1: nc.tensor.matmul(
                      pa[U:128, :], kc0, xT[b1][:, sl], start=False, stop=False)]
            for m, asl in ((0, a1), (1, a2)):
                for j in range(NB):
                    if m == 0 and j == 0:
                        continue
                    def f(pa=pa, p=p, m=m, asl=asl, j=j):
                        nc.tensor.matmul(pa, yc[p][:, j, m, :], asl[:, j, :],
                                         start=False,
                                         stop=(m == 1 and j == NB - 1))
                    th.append(f)
            interleave(th, e_thunks(t, et) if p == 0 else [])
            pas.append(pa)
        for p in range(BL // 2):
            b0, b1 = 2 * p, 2 * p + 1
            pa = pas[p]
            # prefetch h and u for the tail chain
            hp = p3p.tile([128, NTW], F32, name="hp", tag="hp")
            up = p3p.tile([128, NTW], F32, name="up", tag="up")
            for half, b in ((0, b0), (1, b1)):
                psl = slice(half * U, (half + 1) * U)
                nc.sync.dma_start(out=hp[psl, :], in_=d["hT"][b][:, sl])
                nc.sync.dma_start(out=up[psl, :], in_=uscr[b][:, sl])
            pb = psacc.tile([128, NTW], F32, name="pb2", tag="acc")
            for j in range(NB):
                nc.tensor.matmul(pb, yc[p][:, j, 2, :], et[:, j, :],
                                 start=(j == 0), stop=(j == NB - 1))
            tmp = stage.tile([128, NTW], F32, name="tmp2", tag="tmp")
            ct = stage.tile([128, NTW], F32, name="ct", tag="sig")
            t1 = p3p.tile([128, NTW], F32, name="t1", tag="t1")
            # run the gate + elementwise chain in column halves so the
            # DVE / ACT / DMA stages pipeline instead of serializing
            for c0 in range(0, NTW, NTW // 2):
                cs = slice(c0, c0 + NTW // 2)
                nc.vector.tensor_mul(tmp[:, cs], pb[:, cs], rdbc[t][:, cs])
                nc.vector.tensor_add(tmp[:, cs], pa[:, cs], tmp[:, cs])
                nc.scalar.activation(ct[:, cs], tmp[:, cs], AF.Tanh,
                                     scale=0.25, bias=bc2)
                nc.vector.tensor_sub(t1[:, cs], hp[:, cs], ct[:, cs])
                nc.vector.tensor_mul(t1[:, cs], up[:, cs], t1[:, cs])
                nc.vector.tensor_add(t1[:, cs], t1[:, cs], ct[:, cs])
                for half, b in ((0, b0), (1, b1)):
                    psl = slice(half * U, (half + 1) * U)
                    nc.sync.dma_start(
                        out=out_h[b][:, t * NTW + c0:t * NTW + c0 + NTW // 2],
                        in_=t1[psl, cs])

    ctx.close()


_CACHE = {}


def _get_program():
    if "nc" not in _CACHE:
        _CACHE["nc"] = _build_program()
    return _CACHE["nc"]


def _prep_inputs(inputs, h_prev, adj1, adj2, feat, SE, Wq, Wk, Ws1, bs1, Ws2,
                 bs2, r_kernel, r_bias, u_kernel, u_bias, c_kernel, c_bias):
    bf = ml_dtypes.bfloat16
    f32 = np.float32
    perm = list(range(DIN, FROWS)) + list(range(DIN))  # [h(64); inputs(2)]

    h3 = np.asarray(h_prev, f32).reshape(B, N, U)
    hT = np.ascontiguousarray(h3.transpose(0, 2, 1))            # [B, U, N]
    inT = np.asarray(inputs, f32).transpose(0, 2, 1)            # [B, DIN, N]
    xT = np.concatenate([hT, inT], axis=1).astype(bf)           # [B, 66, N]

    rk = np.asarray(r_kernel, f32)[:, perm, :]
    uk = np.asarray(u_kernel, f32)[:, perm, :]
    ck = np.asarray(c_kernel, f32)[:, perm, :]
    kkall = np.concatenate(
        [np.concatenate([rk[m], uk[m]], axis=1) for m in (1, 2, 3)],
        axis=1).astype(bf)                                      # [66, 384]
    kk0 = np.concatenate([rk[0], uk[0]], axis=1).astype(bf)     # [66, 128]
    kcall = np.concatenate([ck[1], ck[2], ck[3]], axis=1).astype(bf)
    kc0 = ck[0].astype(bf)

    shared = {
        "a1T": np.ascontiguousarray(np.asarray(adj1, f32).T).astype(bf),
        "a2T": np.ascontiguousarray(np.asarray(adj2, f32).T).astype(bf),
        "fsT": np.ascontiguousarray(
            np.concatenate([np.asarray(feat, f32).T, np.asarray(SE, f32).T],
                           axis=0)),
        "wq": np.asarray(Wq, f32),
        "wk": np.asarray(Wk, f32),
        "ws1": np.asarray(Ws1, f32),
        "bs1v": np.asarray(bs1, f32).reshape(U, 1),
        "ws2": np.asarray(Ws2, f32).reshape(U, 1),
        "bs2v": np.asarray(bs2, f32).reshape(1, 1),
        "kkall": kkall,
        "kk0": kk0,
        "kcall": kcall,
        "kc0": kc0,
        "bru": np.concatenate([np.asarray(r_bias, f32).mean(0),
                               np.asarray(u_bias, f32).mean(0)]).reshape(-1, 1),
        "bc2": np.tile(np.asarray(c_bias, f32).mean(0), 2).reshape(-1, 1),
    }
    in_maps = []
    for c in range(NCORES):
        bsl = slice(c * BL, (c + 1) * BL)
        m = dict(shared)
        m["xT"] = np.ascontiguousarray(xT[bsl])
        m["hT"] = np.ascontiguousarray(hT[bsl])
        in_maps.append(m)
    return in_maps


def kernel(**inputs):
    os.environ.setdefault("NEURON_RT_RESET_CORES", "1")
    nc = _get_program()
    in_maps = _prep_inputs(**inputs)
    res = None
    err = None
    for _ in range(2):
        try:
            res = run_bass_kernel_spmd(nc, in_maps, list(range(NCORES)))
            break
        except Exception as e:  # e.g. a wedged device; retry once
            err = e
    if res is None:
        raise err
    outs = []
    for c in range(NCORES):
        o = res.results[c]["out"]                     # [BL, U, N] f32
        outs.append(o.transpose(0, 2, 1).reshape(BL, N * U))
    return np.concatenate(outs, axis=0).astype(np.float32)



# revision 4
# speedup vs baseline: 1.9525x; 1.9525x over previous
"""MFGCGRU (graph-conv GRU cell) Trainium2 kernel — fp8 DoubleRow edition.

Strategy: data-parallel over batch B=32 across 8 NeuronCores (4 per core).
The diffusion conv is computed kernel-first (S @ (X k)) with the node
contraction run as fp8e4 DoubleRow matmuls (2 node-blocks of 128 per PE
instruction at 0.5 cycles/row): stationary Y-pair [128, 2, 128] fp8,
moving support-pair [128, 2, 512] fp8.

Precision plan (keeps rel-err ~1.3e-2 < 2e-2 per numpy simulation):
  - x activations, GRU kernels, identity-support matmul, Y-generation all
    bf16; only the big node-contraction operands (supports + Y) are fp8.
  - supports scaled by ALPHA=64 (adj on host; e via exp ln-bias; the
    sentinel row via host-scaled Ws2/bs2), Y scaled by BETA=16 (host-scaled
    kernels); identity kernels carry ALPHA*BETA; sigmoid/tanh scale
    0.25/(ALPHA*BETA) undoes everything.
  - e-support normalizer: d_a = colsum(et) + s_row (both ALPHA-scaled);
    rdbc = ALPHA/d_a broadcast to 128 partitions via a bf16 ones(=ALPHA)
    matmul, so tmp = pb * rdbc lands on the same ALPHA*BETA scale as pa.

et tiles (fp8) and the a1 support slices are cached in SBUF across both
phases; a2 is re-streamed. u is kept in SBUF (bf16) instead of a DRAM
round-trip.
"""

import contextlib
import os

import numpy as np
import ml_dtypes

import concourse.bass as bass
import concourse.bacc as bacc
import concourse.tile as tile
from concourse import mybir
from concourse.bass_utils import run_bass_kernel_spmd

F32 = mybir.dt.float32
BF16 = mybir.dt.bfloat16
FP8 = mybir.dt.float8e4
AF = mybir.ActivationFunctionType
DR = mybir.MatmulPerfMode.DoubleRow

B, N, DIN, U, FD, SD = 32, 2048, 2, 64, 32, 64
NCORES = 8
BL = B // NCORES          # batches per core
NTW = 512                 # n-tile width (output columns per tile)
NT = N // NTW             # 4 n-tiles
NBW = 128                 # node-block width
NB = N // NBW             # 16 node blocks
NJP = NB // 2             # 8 node-block pairs (DoubleRow)
FROWS = DIN + U           # 66

ALPHA = 64.0              # support scale
BETA = 16.0               # Y scale
GSCALE = 0.25 / (ALPHA * BETA)   # gate pre-activation descale (incl /M)


def _build_program():
    nc = bacc.Bacc("TRN2", debug=False, num_devices=NCORES)

    d = {}

    def din(name, shape, dt):
        d[name] = nc.dram_tensor(name, shape, dt, kind="ExternalInput").ap()

    din("xT", [BL, FROWS, N], BF16)
    din("hT", [BL, U, N], BF16)
    din("a1q", [NT, NBW, NB, NTW], FP8)
    din("a2q", [NT, NBW, NB, NTW], FP8)
    din("fsT", [FD + SD, N], BF16)
    din("wq", [FD, U], BF16)
    din("wk", [FD, U], BF16)
    din("ws1", [FD + SD, U], BF16)
    din("bs1v", [U, 1], F32)
    din("ws2a", [U, 1], BF16)
    din("bs2a", [1, 1], F32)
    din("kkall", [FROWS, 3 * 2 * U], BF16)
    din("kk0", [FROWS, 2 * U], BF16)
    din("kcall", [FROWS, 3 * U], BF16)
    din("kc0", [FROWS, U], BF16)
    din("bru", [2 * U, 1], F32)
    din("bc2", [2 * U, 1], F32)
    out_h = nc.dram_tensor("out", [BL, U, N], F32, kind="ExternalOutput").ap()

    with tile.TileContext(nc) as tc:
        _emit(tc, d, out_h)
    nc.compile()
    return nc


def _emit(tc, d, out_h):
    nc = tc.nc
    ctx = contextlib.ExitStack()
    const = ctx.enter_context(tc.tile_pool(name="const", bufs=1))
    persist = ctx.enter_context(tc.tile_pool(name="persist", bufs=1))
    a2p = ctx.enter_context(tc.tile_pool(name="a2p", bufs=2))
    stage = ctx.enter_context(tc.tile_pool(name="stage", bufs=2))
    p3p = ctx.enter_context(tc.tile_pool(name="p3p", bufs=2))
    psacc = ctx.enter_context(tc.tile_pool(name="psacc", bufs=4, space="PSUM"))
    psscr = ctx.enter_context(tc.tile_pool(name="psscr", bufs=3, space="PSUM"))

    # ---- constants / weights in SBUF ----
    def cload(name):
        ap = d[name]
        t = const.tile(list(ap.shape), ap.dtype, name=f"c_{name}")
        nc.sync.dma_start(out=t, in_=ap)
        return t

    fsT = cload("fsT")
    wq = cload("wq")
    wk = cload("wk")
    ws1 = cload("ws1")
    bs1v = cload("bs1v")
    ws2a = cload("ws2a")
    bs2a = cload("bs2a")
    kkall = cload("kkall")
    kk0 = cload("kk0")
    kcall = cload("kcall")
    kc0 = cload("kc0")
    bru = cload("bru")
    bc2 = cload("bc2")

    ones2 = const.tile([NBW, 2, 16], FP8, name="ones2")
    nc.vector.memset(ones2, 1.0)
    ones_row = const.tile([1, NBW], BF16, name="ones_row")
    nc.vector.memset(ones_row, ALPHA)
    lnal = const.tile([NBW, 1], F32, name="lnal")
    nc.vector.memset(lnal, float(np.log(ALPHA)))

    # ---- persistent activations ----
    xT = [persist.tile([FROWS, N], BF16, name=f"xT{b}", tag=f"xT{b}")
          for b in range(BL)]
    for b in range(BL):
        nc.sync.dma_start(out=xT[b], in_=d["xT"][b])

    QT = persist.tile([FD, 2, N], FP8, name="QT", tag="QT")
    KT = persist.tile([FD, 2, N], FP8, name="KT", tag="KT")
    s_row = persist.tile([1, N], F32, name="s_row", tag="s_row")
    rdbc = [persist.tile([NBW, NTW], BF16, name=f"rdbc{t}", tag=f"rdbc{t}")
            for t in range(NT)]
    et = [persist.tile([NBW, NB, NTW], FP8, name=f"et{t}", tag=f"et{t}")
          for t in range(NT)]
    a1c = [persist.tile([NBW, NB, NTW], FP8, name=f"a1c{t}", tag=f"a1c{t}")
           for t in range(NT)]
    ut = [persist.tile([NBW, N], BF16, name=f"ut{p}", tag=f"ut{p}")
          for p in range(BL // 2)]
    y = [persist.tile([NBW, NB, 3, 2 * U], FP8, name=f"y_{b}", tag=f"y{b}")
         for b in range(BL)]
    yc = [persist.tile([NBW, NB, 3, 2 * U], FP8, name=f"yc_{p}", tag=f"yc{p}")
          for p in range(BL // 2)]

    # adjacency DMAs for phase-1 t=0 (a1 is cached for both phases; a2
    # streamed per phase)
    nc.sync.dma_start(out=a1c[0], in_=d["a1q"][0])
    a2t = [None] * NT
    a2t[0] = a2p.tile([NBW, NB, NTW], FP8, name="a2_0", tag="a2")
    nc.sync.dma_start(out=a2t[0], in_=d["a2q"][0])

    # ---- prelude: Q^T, K^T (fp8, u-split layout), s_row ----
    for t in range(NT):
        sl = slice(t * NTW, (t + 1) * NTW)
        for dst, w in ((QT, wq), (KT, wk)):
            for uh in range(2):
                pq = psscr.tile([FD, NTW], F32, name="pq", tag="scr")
                nc.tensor.matmul(pq, w[:, uh * FD:(uh + 1) * FD],
                                 fsT[0:FD, sl], start=True, stop=True)
                nc.scalar.activation(dst[:, uh, sl], pq, AF.Relu)
        ps1 = psscr.tile([U, NTW], F32, name="ps1", tag="scr")
        nc.tensor.matmul(ps1, ws1, fsT[:, sl], start=True, stop=True)
        s1t = stage.tile([U, NTW], BF16, name="s1t", tag="s1t")
        nc.scalar.activation(s1t, ps1, AF.Relu, bias=bs1v)
        ps2 = psscr.tile([1, NTW], F32, name="ps2", tag="scr")
        nc.tensor.matmul(ps2, ws2a, s1t, start=True, stop=True)
        nc.scalar.activation(s_row[:, sl], ps2, AF.Relu, bias=bs2a)

    # ---- Y tiles for r/u: y[b] = beta * X_b @ [k_r[m] | k_u[m]] ----
    # stored [node%128, node//128, m, u'] fp8; evacuation alternates DVE/ACT
    for b in range(BL):
        for j in range(NB):
            nsl = slice(j * NBW, (j + 1) * NBW)
            py = psscr.tile([NBW, 3 * 2 * U], F32, name="py", tag="scr")
            nc.tensor.matmul(py, xT[b][:, nsl], kkall, start=True, stop=True)
            src = py.rearrange("p (m u) -> p m u", m=3)
            if (b * NB + j) % 2 == 0:
                nc.vector.tensor_copy(y[b][:, j, :, :], src)
            else:
                nc.scalar.activation(y[b][:, j, :, :], src, AF.Copy)

    def ycgen(p):
        for half in range(2):
            b = 2 * p + half
            usl = slice(half * U, (half + 1) * U)
            for j in range(NB):
                nsl = slice(j * NBW, (j + 1) * NBW)
                pyc = psscr.tile([NBW, 3 * U], F32, name="pyc", tag="scr")
                nc.tensor.matmul(pyc, xT[b][:, nsl], kcall,
                                 start=True, stop=True)
                src = pyc.rearrange("p (m u) -> p m u", m=3)
                if j % 2 == 0:
                    nc.vector.tensor_copy(yc[p][:, j, :, usl], src)
                else:
                    nc.scalar.activation(yc[p][:, j, :, usl], src, AF.Copy)

    # =================== phase 1: r & u gates ===================
    for t in range(NT):
        sl = slice(t * NTW, (t + 1) * NTW)
        a1, a2 = a1c[t], a2t[t]

        # e^T tile gen: et[t][:, j, :] = ALPHA * exp(K^T Q / 8), fp8
        for j in range(NB):
            pe = psscr.tile([NBW, NTW], F32, name="pe", tag="scr")
            nc.tensor.matmul(pe, KT[:, :, j * NBW:(j + 1) * NBW],
                             QT[:, :, sl], start=True, stop=True, perf_mode=DR)
            nc.scalar.activation(et[t][:, j, :], pe, AF.Exp, scale=0.125,
                                 bias=lnal)

        # A-groups: pa[b] = ALPHA*BETA * (identity + a1 + a2 contributions)
        pas = []
        for b in range(BL):
            pa = psacc.tile([NBW, NTW], F32, name="pa", tag="acc")
            nc.tensor.matmul(pa, kk0, xT[b][:, sl], start=True, stop=False)
            for m, asl in ((0, a1), (1, a2)):
                for jp in range(NJP):
                    js = slice(2 * jp, 2 * jp + 2)
                    nc.tensor.matmul(pa, y[b][:, js, m, :], asl[:, js, :],
                                     start=False,
                                     stop=(m == 1 and jp == NJP - 1),
                                     perf_mode=DR)
            pas.append(pa)

        # prefetch next tile's adjacency
        if t + 1 < NT:
            nc.sync.dma_start(out=a1c[t + 1], in_=d["a1q"][t + 1])
            a2t[t + 1] = a2p.tile([NBW, NB, NTW], FP8, name=f"a2_{t+1}",
                                  tag="a2")
            nc.sync.dma_start(out=a2t[t + 1], in_=d["a2q"][t + 1])

        # d_a = colsum(et) + s_row  (both ALPHA-scaled);  rdbc = ALPHA/d_a
        pd = psscr.tile([1, NTW], F32, name="pd", tag="scr")
        for jp in range(NJP):
            js = slice(2 * jp, 2 * jp + 2)
            nc.tensor.matmul(pd, ones2[:, :, 0:1], et[t][:, js, :],
                             start=(jp == 0), stop=(jp == NJP - 1),
                             perf_mode=DR)
        dsb = stage.tile([1, NTW], F32, name="dsb", tag="dsb")
        nc.vector.tensor_add(dsb, pd, s_row[:, sl])
        rds = stage.tile([1, NTW], BF16, name="rds", tag="dsb")
        with nc.allow_low_precision(reason="rdbc is bf16 by design"):
            nc.vector.reciprocal(rds, dsb)
        pr = psscr.tile([NBW, NTW], F32, name="pr", tag="scr")
        nc.tensor.matmul(pr, ones_row, rds, start=True, stop=True)
        nc.scalar.activation(rdbc[t], pr, AF.Copy)

        # B-groups + epilogue
        for b in range(BL):
            pb = psscr.tile([NBW, NTW], F32, name="pb", tag="scr")
            for jp in range(NJP):
                js = slice(2 * jp, 2 * jp + 2)
                nc.tensor.matmul(pb, y[b][:, js, 2, :], et[t][:, js, :],
                                 start=(jp == 0), stop=(jp == NJP - 1),
                                 perf_mode=DR)
            tmp = stage.tile([NBW, NTW], BF16, name="tmp", tag="tmp")
            nc.vector.tensor_mul(tmp, pb, rdbc[t])
            ssum = stage.tile([NBW, NTW], BF16, name="ssum", tag="ssum")
            nc.vector.tensor_add(ssum, pas[b], tmp)
            # r rows 0:64 -> multiply into xT h-rows; u rows 64:128 -> ut
            rt = stage.tile([U, NTW], BF16, name="rt", tag="rt")
            nc.scalar.activation(rt, ssum[0:U, :], AF.Sigmoid,
                                 scale=GSCALE, bias=bru[0:U, :])
            p, half = b // 2, b % 2
            nc.scalar.activation(ut[p][half * U:(half + 1) * U, sl],
                                 ssum[U:2 * U, :], AF.Sigmoid,
                                 scale=GSCALE, bias=bru[U:2 * U, :])
            nc.vector.tensor_mul(xT[b][0:U, sl], rt, xT[b][0:U, sl])

    # =================== phase 2: c gate & h_new ===================
    ycgen(0)
    ycgen(1)

    a2t2 = [None] * NT
    a2t2[0] = a2p.tile([NBW, NB, NTW], FP8, name="a2b_0", tag="a2")
    nc.sync.dma_start(out=a2t2[0], in_=d["a2q"][0])

    for t in range(NT):
        sl = slice(t * NTW, (t + 1) * NTW)
        a1, a2 = a1c[t], a2t2[t]

        hps = []
        for p in range(BL // 2):
            hp = p3p.tile([NBW, NTW], BF16, name="hp", tag="hp")
            for half in range(2):
                b = 2 * p + half
                nc.sync.dma_start(out=hp[half * U:(half + 1) * U, :],
                                  in_=d["hT"][b][:, sl])
            hps.append(hp)

        pas = []
        for p in range(BL // 2):
            b0, b1 = 2 * p, 2 * p + 1
            pa = psacc.tile([NBW, NTW], F32, name="pa2", tag="acc")
            # first matmul of the group must span all 128 partitions with
            # start=True (start zeroes the whole bank region); the two
            # half-partition identity matmuls then accumulate.
            nc.tensor.matmul(pa, yc[p][:, 0:2, 0, :], a1[:, 0:2, :],
                             start=True, stop=False, perf_mode=DR)
            nc.tensor.matmul(pa[0:U, :], kc0, xT[b0][:, sl],
                             start=False, stop=False)
            nc.tensor.matmul(pa[U:2 * U, :], kc0, xT[b1][:, sl],
                             start=False, stop=False)
            for m, asl in ((0, a1), (1, a2)):
                for jp in range(NJP):
                    if m == 0 and jp == 0:
                        continue
                    js = slice(2 * jp, 2 * jp + 2)
                    nc.tensor.matmul(pa, yc[p][:, js, m, :], asl[:, js, :],
                                     start=False,
                                     stop=(m == 1 and jp == NJP - 1),
                                     perf_mode=DR)
            pas.append(pa)

        if t + 1 < NT:
            a2t2[t + 1] = a2p.tile([NBW, NB, NTW], FP8, name=f"a2b_{t+1}",
                                   tag="a2")
            nc.sync.dma_start(out=a2t2[t + 1], in_=d["a2q"][t + 1])

        for p in range(BL // 2):
            b0, b1 = 2 * p, 2 * p + 1
            pb = psscr.tile([NBW, NTW], F32, name="pb2", tag="scr")
            for jp in range(NJP):
                js = slice(2 * jp, 2 * jp + 2)
                nc.tensor.matmul(pb, yc[p][:, js, 2, :], et[t][:, js, :],
                                 start=(jp == 0), stop=(jp == NJP - 1),
                                 perf_mode=DR)
            tmp = stage.tile([NBW, NTW], BF16, name="tmp2", tag="tmp")
            nc.vector.tensor_mul(tmp, pb, rdbc[t])
            ssum = stage.tile([NBW, NTW], BF16, name="ssum2", tag="ssum")
            nc.vector.tensor_add(ssum, pas[p], tmp)
            ct = stage.tile([NBW, NTW], BF16, name="ct", tag="ct")
            nc.scalar.activation(ct, ssum, AF.Tanh, scale=GSCALE, bias=bc2)
            # h_new = u*h + (1-u)*c = (h - c)*u + c
            q = stage.tile([NBW, NTW], BF16, name="q", tag="q")
            nc.vector.tensor_sub(q, hps[p], ct)
            nc.vector.tensor_mul(q, q, ut[p][:, sl])
            t1 = p3p.tile([NBW, NTW], F32, name="t1", tag="t1")
            nc.vector.tensor_add(t1, q, ct)
            for half, b in ((0, b0), (1, b1)):
                psl = slice(half * U, (half + 1) * U)
                nc.sync.dma_start(out=out_h[b][:, sl], in_=t1[psl, :])

    ctx.close()


_CACHE = {}


def _get_program():
    if "nc" not in _CACHE:
        _CACHE["nc"] = _build_program()
    return _CACHE["nc"]


def _prep_inputs(inputs, h_prev, adj1, adj2, feat, SE, Wq, Wk, Ws1, bs1, Ws2,
                 bs2, r_kernel, r_bias, u_kernel, u_bias, c_kernel, c_bias):
    bf = ml_dtypes.bfloat16
    f8 = ml_dtypes.float8_e4m3
    f32 = np.float32
    perm = list(range(DIN, FROWS)) + list(range(DIN))  # [h(64); inputs(2)]

    h3 = np.asarray(h_prev, f32).reshape(B, N, U)
    hT = np.ascontiguousarray(h3.transpose(0, 2, 1))            # [B, U, N]
    inT = np.asarray(inputs, f32).transpose(0, 2, 1)            # [B, DIN, N]
    xT = np.concatenate([hT, inT], axis=1).astype(bf)           # [B, 66, N]
    hTb = hT.astype(bf)

    rk = np.asarray(r_kernel, f32)[:, perm, :]
    uk = np.asarray(u_kernel, f32)[:, perm, :]
    ck = np.asarray(c_kernel, f32)[:, perm, :]
    kkall = (BETA * np.concatenate(
        [np.concatenate([rk[m], uk[m]], axis=1) for m in (1, 2, 3)],
        axis=1)).astype(bf)                                     # [66, 384]
    kk0 = (ALPHA * BETA * np.concatenate([rk[0], uk[0]], axis=1)).astype(bf)
    kcall = (BETA * np.concatenate([ck[1], ck[2], ck[3]], axis=1)).astype(bf)
    kc0 = (ALPHA * BETA * ck[0]).astype(bf)

    def adj_tiles(a):
        aT = np.asarray(a, f32).T * ALPHA
        np.clip(aT, -240.0, 240.0, out=aT)
        q = aT.astype(f8)                                      # [n, m] = A^T
        # tile[t, p, j, w] = A^T[j*128 + p, t*512 + w]
        return np.ascontiguousarray(
            q.reshape(NB, NBW, NT, NTW).transpose(2, 1, 0, 3))

    shared = {
        "a1q": adj_tiles(adj1),
        "a2q": adj_tiles(adj2),
        "fsT": np.ascontiguousarray(
            np.concatenate([np.asarray(feat, f32).T, np.asarray(SE, f32).T],
                           axis=0)).astype(bf),
        "wq": np.asarray(Wq, f32).astype(bf),
        "wk": np.asarray(Wk, f32).astype(bf),
        "ws1": np.asarray(Ws1, f32).astype(bf),
        "bs1v": np.asarray(bs1, f32).reshape(U, 1),
        "ws2a": (ALPHA * np.asarray(Ws2, f32)).reshape(U, 1).astype(bf),
        "bs2a": (ALPHA * np.asarray(bs2, f32)).reshape(1, 1),
        "kkall": kkall,
        "kk0": kk0,
        "kcall": kcall,
        "kc0": kc0,
        "bru": np.concatenate([np.asarray(r_bias, f32).mean(0),
                               np.asarray(u_bias, f32).mean(0)]).reshape(-1, 1),
        "bc2": np.tile(np.asarray(c_bias, f32).mean(0), 2).reshape(-1, 1),
    }
    in_maps = []
    for c in range(NCORES):
        bsl = slice(c * BL, (c + 1) * BL)
        m = dict(shared)
        m["xT"] = np.ascontiguousarray(xT[bsl])
        m["hT"] = np.ascontiguousarray(hTb[bsl])
        in_maps.append(m)
    return in_maps


def kernel(**inputs):
    os.environ.setdefault("NEURON_RT_RESET_CORES", "1")
    nc = _get_program()
    in_maps = _prep_inputs(**inputs)
    res = None
    err = None
    for _ in range(2):
        try:
            res = run_bass_kernel_spmd(nc, in_maps, list(range(NCORES)))
            break
        except Exception as e:  # e.g. a wedged device; retry once
            err = e
    if res is None:
        raise err
    outs = []
    for c in range(NCORES):
        o = res.results[c]["out"]                     # [BL, U, N] f32
        outs.append(o.transpose(0, 2, 1).reshape(BL, N * U))
    return np.concatenate(outs, axis=0).astype(np.float32)


# revision 9
# speedup vs baseline: 1.9557x; 1.0017x over previous
"""MFGCGRU (graph-conv GRU cell) Trainium2 kernel — fp8 DoubleRow edition.

Strategy: data-parallel over batch B=32 across 8 NeuronCores (4 per core).
The diffusion conv is computed kernel-first (S @ (X k)) with the node
contraction run as fp8e4 DoubleRow matmuls (2 node-blocks of 128 per PE
instruction at 0.5 cycles/row): stationary Y-pair [128, 2, 128] fp8,
moving support-pair [128, 2, 512] fp8 (1024-wide moving).

Precision plan (rel-err ~1.3e-2 < 2e-2 on HW):
  - x activations, GRU kernels, identity-support matmul, Y-generation all
    bf16; only the big node-contraction operands (supports + Y) are fp8.
  - supports scaled by ALPHA=64 (adj on host; e via exp ln-bias; the
    sentinel row via host-scaled Ws2/bs2), Y scaled by BETA=16 (host-scaled
    kernels); identity kernels carry ALPHA*BETA; the gate activations
    descale by 0.25/(ALPHA*BETA).

Scheduling: the whole program uses only the `exp_and_others` activation
table — both GRU sigmoids are computed as (1+tanh(z/2))/2 with the /2 and
+1 folded into host-side constants — so the ACT engine never reloads its
function table.  e^T generation for tile t+1 is interleaved into tile t's
A-group matmuls so the PE never waits on the ACT exp evacuations; et tiles
(fp8) and the a1 support slices are cached in SBUF across both phases;
u (as tanh) is kept in SBUF instead of a DRAM round-trip.
"""

import contextlib
import os

import numpy as np
import ml_dtypes

import concourse.bass as bass
import concourse.bacc as bacc
import concourse.tile as tile
from concourse import mybir
from concourse.bass_utils import run_bass_kernel_spmd

F32 = mybir.dt.float32
BF16 = mybir.dt.bfloat16
FP8 = mybir.dt.float8e4
AF = mybir.ActivationFunctionType
DR = mybir.MatmulPerfMode.DoubleRow
OP = mybir.AluOpType

B, N, DIN, U, FD, SD = 32, 2048, 2, 64, 32, 64
NCORES = 8
BL = B // NCORES          # batches per core
NTW = 512                 # n-tile width (output columns per tile)
NT = N // NTW             # 4 n-tiles
NBW = 128                 # node-block width
NB = N // NBW             # 16 node blocks
NJP = NB // 2             # 8 node-block pairs (DoubleRow)
FROWS = DIN + U           # 66

ALPHA = 64.0              # support scale
BETA = 16.0               # Y scale
GSCALE = 0.25 / (ALPHA * BETA)   # gate pre-activation descale (incl /M)
GS2 = GSCALE / 2.0               # tanh-form sigmoid input scale


def _build_program():
    nc = bacc.Bacc("TRN2", debug=False, num_devices=NCORES)

    d = {}

    def din(name, shape, dt):
        d[name] = nc.dram_tensor(name, shape, dt, kind="ExternalInput").ap()

    din("xT", [BL, FROWS, N], BF16)
    din("hT", [BL, U, N], BF16)        # pre-scaled by 0.5 on host
    din("a1q", [NT, NBW, NB, NTW], FP8)
    din("a2q", [NT, NBW, NB, NTW], FP8)
    din("fsT", [FD + SD, N], BF16)
    din("wq", [FD, U], BF16)
    din("wk", [FD, U], BF16)
    din("ws1", [FD + SD, U], BF16)
    din("bs1v", [U, 1], F32)
    din("ws2a", [U, 1], BF16)
    din("bs2a", [1, 1], F32)
    din("kkall", [FROWS, 3 * 2 * U], BF16)
    din("kk0", [FROWS, 2 * U], BF16)
    din("kcall", [FROWS, 3 * U], BF16)
    din("kc0", [FROWS, U], BF16)
    din("bru2", [2 * U, 1], F32)
    din("bc2", [2 * U, 1], F32)
    out_h = nc.dram_tensor("out", [BL, U, N], BF16, kind="ExternalOutput").ap()

    with tile.TileContext(nc) as tc:
        _emit(tc, d, out_h)
    nc.compile()
    return nc


def _interleave(main, extra, ratio=4):
    """Emit `ratio` thunks from main per one from extra."""
    mi = ei = 0
    while mi < len(main) or ei < len(extra):
        for _ in range(ratio):
            if mi < len(main):
                main[mi]()
                mi += 1
        if ei < len(extra):
            extra[ei]()
            ei += 1


def _emit(tc, d, out_h):
    nc = tc.nc
    ctx = contextlib.ExitStack()
    const = ctx.enter_context(tc.tile_pool(name="const", bufs=1))
    persist = ctx.enter_context(tc.tile_pool(name="persist", bufs=1))
    a2p = ctx.enter_context(tc.tile_pool(name="a2p", bufs=2))
    stage = ctx.enter_context(tc.tile_pool(name="stage", bufs=2))
    p3p = ctx.enter_context(tc.tile_pool(name="p3p", bufs=2))
    psacc = ctx.enter_context(tc.tile_pool(name="psacc", bufs=4, space="PSUM"))
    psscr = ctx.enter_context(tc.tile_pool(name="psscr", bufs=2, space="PSUM"))
    psb = ctx.enter_context(tc.tile_pool(name="psb", bufs=2, space="PSUM"))

    # ---- constants / weights in SBUF ----
    def cload(name):
        ap = d[name]
        t = const.tile(list(ap.shape), ap.dtype, name=f"c_{name}")
        nc.sync.dma_start(out=t, in_=ap)
        return t

    fsT = cload("fsT")
    wq = cload("wq")
    wk = cload("wk")
    ws1 = cload("ws1")
    bs1v = cload("bs1v")
    ws2a = cload("ws2a")
    bs2a = cload("bs2a")
    kkall = cload("kkall")
    kk0 = cload("kk0")
    kcall = cload("kcall")
    kc0 = cload("kc0")
    bru2 = cload("bru2")
    bc2 = cload("bc2")

    ones2 = const.tile([NBW, 2, 16], FP8, name="ones2")
    nc.vector.memset(ones2, 1.0)
    ones_row = const.tile([1, NBW], BF16, name="ones_row")
    nc.vector.memset(ones_row, ALPHA)
    lnal = const.tile([NBW, 1], F32, name="lnal")
    nc.vector.memset(lnal, float(np.log(ALPHA)))

    # ---- persistent activations ----
    xT = [persist.tile([FROWS, N], BF16, name=f"xT{b}", tag=f"xT{b}")
          for b in range(BL)]
    for b in range(BL):
        nc.sync.dma_start(out=xT[b], in_=d["xT"][b])

    QT = persist.tile([FD, 2, N], FP8, name="QT", tag="QT")
    KT = persist.tile([FD, 2, N], FP8, name="KT", tag="KT")
    s_row = persist.tile([1, N], F32, name="s_row", tag="s_row")
    rdbc = [persist.tile([NBW, NTW], BF16, name=f"rdbc{t}", tag=f"rdbc{t}")
            for t in range(NT)]
    et = [persist.tile([NBW, NB, NTW], FP8, name=f"et{t}", tag=f"et{t}")
          for t in range(NT)]
    a1c = [persist.tile([NBW, NB, NTW], FP8, name=f"a1c{t}", tag=f"a1c{t}")
           for t in range(NT)]
    ut = [persist.tile([NBW, N], BF16, name=f"ut{p}", tag=f"ut{p}")
          for p in range(BL // 2)]
    y = [persist.tile([NBW, NB, 3, 2 * U], FP8, name=f"y_{b}", tag=f"y{b}")
         for b in range(BL)]
    yc = [persist.tile([NBW, NB, 3, 2 * U], FP8, name=f"yc_{p}", tag=f"yc{p}")
          for p in range(BL // 2)]

    # adjacency DMAs for phase-1 t=0 (a1 cached for both phases; a2 streamed)
    nc.sync.dma_start(out=a1c[0], in_=d["a1q"][0])
    a2t = [None] * NT
    a2t[0] = a2p.tile([NBW, NB, NTW], FP8, name="a2_0", tag="a2")
    nc.sync.dma_start(out=a2t[0], in_=d["a2q"][0])

    # ---- thunk generators ----
    def prelude_thunks():
        th = []
        for t in range(NT):
            sl = slice(t * NTW, (t + 1) * NTW)
            # KT before QT: the interleaved e-gen(t0) thunk ei fires after
            # main[4*ei+3]; ei=0 needs all four K/Q writes of the t0 block
            # already emitted (Tile cannot depend on future instructions)
            for dst, w in ((KT, wk), (QT, wq)):
                for uh in range(2):
                    def f(dst=dst, w=w, uh=uh, sl=sl):
                        pq = psscr.tile([FD, NTW], F32, name="pq", tag="scr")
                        nc.tensor.matmul(pq, w[:, uh * FD:(uh + 1) * FD],
                                         fsT[0:FD, sl], start=True, stop=True)
                        # split QK relu evacs between ACT and DVE
                        if uh == 0:
                            nc.scalar.activation(dst[:, uh, sl], pq, AF.Relu)
                        else:
                            nc.vector.tensor_scalar_max(dst[:, uh, sl], pq, 0.0)
                    th.append(f)

            def g(sl=sl):
                ps1 = psscr.tile([U, NTW], F32, name="ps1", tag="scr")
                nc.tensor.matmul(ps1, ws1, fsT[:, sl], start=True, stop=True)
                s1t = stage.tile([U, NTW], BF16, name="s1t", tag="s1t")
                nc.scalar.activation(s1t, ps1, AF.Relu, bias=bs1v)
                ps2 = psscr.tile([1, NTW], F32, name="ps2", tag="scr")
                nc.tensor.matmul(ps2, ws2a, s1t, start=True, stop=True)
                nc.scalar.activation(s_row[:, sl], ps2, AF.Relu, bias=bs2a)
            th.append(g)
        return th

    def ygen_thunks(b):
        th = []
        for j in range(NB):
            def f(b=b, j=j):
                nsl = slice(j * NBW, (j + 1) * NBW)
                py = psscr.tile([NBW, 3 * 2 * U], F32, name="py", tag="scr")
                nc.tensor.matmul(py, xT[b][:, nsl], kkall,
                                 start=True, stop=True)
                src = py.rearrange("p (m u) -> p m u", m=3)
                if (b * NB + j) % 16 < 11:
                    nc.vector.tensor_copy(y[b][:, j, :, :], src)
                else:
                    nc.scalar.activation(y[b][:, j, :, :], src, AF.Copy)
            th.append(f)
        return th

    def ycgen_thunks(p):
        th = []
        for half in range(2):
            b = 2 * p + half
            usl = slice(half * U, (half + 1) * U)
            for j0 in range(0, NB, 2):
                def f(b=b, usl=usl, j0=j0, p=p):
                    pyc = psscr.tile([NBW, 2 * 3 * U], F32, name="pyc",
                                     tag="scr")
                    for i in range(2):
                        nsl = slice((j0 + i) * NBW, (j0 + i + 1) * NBW)
                        # start zeroes the whole PSUM bank region: only the
                        # first matmul of the pair may set it
                        nc.tensor.matmul(pyc[:, i * 3 * U:(i + 1) * 3 * U],
                                         xT[b][:, nsl], kcall,
                                         start=(i == 0), stop=(i == 1))
                    src = pyc.rearrange("p (j m u) -> p j m u", j=2, m=3)
                    if (b * NB + j0) % 4 < 2:
                        nc.vector.tensor_copy(yc[p][:, j0:j0 + 2, :, usl], src)
                    else:
                        nc.scalar.activation(yc[p][:, j0:j0 + 2, :, usl], src,
                                             AF.Copy)
                th.append(f)
        return th

    def egen_thunks(t):
        sl = slice(t * NTW, (t + 1) * NTW)
        th = []
        for j in range(NB):
            def f(j=j, t=t, sl=sl):
                pe = psscr.tile([NBW, NTW], F32, name="pe", tag="scr")
                nc.tensor.matmul(pe, KT[:, :, j * NBW:(j + 1) * NBW],
                                 QT[:, :, sl], start=True, stop=True,
                                 perf_mode=DR)
                nc.scalar.activation(et[t][:, j, :], pe, AF.Exp, scale=0.125,
                                     bias=lnal)
            th.append(f)
        return th

    def agroup1_thunks(b, t, a1, a2, pa):
        sl = slice(t * NTW, (t + 1) * NTW)
        th = [lambda: nc.tensor.matmul(pa, kk0, xT[b][:, sl],
                                       start=True, stop=False)]
        for m, asl in ((0, a1), (1, a2)):
            for jp in range(NJP):
                def f(m=m, asl=asl, jp=jp, b=b):
                    js = slice(2 * jp, 2 * jp + 2)
                    nc.tensor.matmul(pa, y[b][:, js, m, :], asl[:, js, :],
                                     start=False,
                                     stop=(m == 1 and jp == NJP - 1),
                                     perf_mode=DR)
                th.append(f)
        return th

    def dblock(t):
        sl = slice(t * NTW, (t + 1) * NTW)
        pd = psb.tile([1, NTW], F32, name="pd", tag="pb")
        for jp in range(NJP):
            js = slice(2 * jp, 2 * jp + 2)
            nc.tensor.matmul(pd, ones2[:, :, 0:1], et[t][:, js, :],
                             start=(jp == 0), stop=(jp == NJP - 1),
                             perf_mode=DR)
        dsb = stage.tile([1, NTW], F32, name="dsb", tag="dsb")
        nc.vector.tensor_add(dsb, pd, s_row[:, sl])
        rds = stage.tile([1, NTW], BF16, name="rds", tag="dsb")
        with nc.allow_low_precision(reason="rdbc is bf16 by design"):
            nc.vector.reciprocal(rds, dsb)
        pr = psb.tile([NBW, NTW], F32, name="pr", tag="pb")
        nc.tensor.matmul(pr, ones_row, rds, start=True, stop=True)
        nc.scalar.activation(rdbc[t], pr, AF.Copy)

    def bgroup1(b, t, pa):
        sl = slice(t * NTW, (t + 1) * NTW)
        pb = psb.tile([NBW, NTW], F32, name="pb", tag="pb")
        for jp in range(NJP):
            js = slice(2 * jp, 2 * jp + 2)
            nc.tensor.matmul(pb, y[b][:, js, 2, :], et[t][:, js, :],
                             start=(jp == 0), stop=(jp == NJP - 1),
                             perf_mode=DR)
        tmp = stage.tile([NBW, NTW], BF16, name="tmp", tag="tmp")
        nc.vector.tensor_mul(tmp, pb, rdbc[t])
        ssum = stage.tile([NBW, NTW], BF16, name="ssum", tag="ssum")
        nc.vector.tensor_add(ssum, pa, tmp)
        # th = tanh(GS2*ssum + bru/2): rows 0:64 -> (1+th)*h into xT h-rows
        # (factor 2 absorbed in host-halved kcall/kc0 h-rows); rows 64:128
        # -> ut stores tanh-form u
        th = stage.tile([NBW, NTW], BF16, name="th", tag="th")
        nc.scalar.activation(th, ssum, AF.Tanh, scale=GS2, bias=bru2)
        nc.vector.scalar_tensor_tensor(
            xT[b][0:U, sl], th[0:U, :], 1.0, xT[b][0:U, sl],
            op0=OP.add, op1=OP.mult)
        p, half = b // 2, b % 2
        nc.vector.tensor_copy(ut[p][half * U:(half + 1) * U, sl],
                              th[U:2 * U, :])

    # ==================== pre-phase ====================
    pre = prelude_thunks() + ygen_thunks(0) + ygen_thunks(1)
    _interleave(pre, egen_thunks(0), ratio=4)

    # ==================== phase 1: r & u gates ====================
    for t in range(NT):
        sl = slice(t * NTW, (t + 1) * NTW)
        a1, a2 = a1c[t], a2t[t]

        pas = []
        amain = []
        for b in range(BL):
            pa = psacc.tile([NBW, NTW], F32, name="pa", tag="acc")
            amain.extend(agroup1_thunks(b, t, a1, a2, pa))
            pas.append(pa)
            if t == 0 and b == 0:
                amain.extend(ygen_thunks(2))
            if t == 0 and b == 1:
                amain.extend(ygen_thunks(3))
        _interleave(amain, egen_thunks(t + 1) if t + 1 < NT else [], ratio=4)

        if t + 1 < NT:
            nc.sync.dma_start(out=a1c[t + 1], in_=d["a1q"][t + 1])
            a2t[t + 1] = a2p.tile([NBW, NB, NTW], FP8, name=f"a2_{t+1}",
                                  tag="a2")
            nc.sync.dma_start(out=a2t[t + 1], in_=d["a2q"][t + 1])

        dblock(t)
        for b in range(BL):
            bgroup1(b, t, pas[b])

    # ==================== phase 2: c gate & h_new ====================
    a2t2 = [None] * NT
    a2t2[0] = a2p.tile([NBW, NB, NTW], FP8, name="a2b_0", tag="a2")
    nc.sync.dma_start(out=a2t2[0], in_=d["a2q"][0])

    for f in ycgen_thunks(0):
        f()
    for f in ycgen_thunks(1):
        f()

    for t in range(NT):
        sl = slice(t * NTW, (t + 1) * NTW)
        a1, a2 = a1c[t], a2t2[t]

        hps = []
        for p in range(BL // 2):
            hp = p3p.tile([NBW, NTW], BF16, name="hp", tag="hp")
            for half in range(2):
                b = 2 * p + half
                nc.sync.dma_start(out=hp[half * U:(half + 1) * U, :],
                                  in_=d["hT"][b][:, sl])
            hps.append(hp)

        pas = []
        for p in range(BL // 2):
            b0, b1 = 2 * p, 2 * p + 1
            pa = psacc.tile([NBW, NTW], F32, name="pa2", tag="acc")
            # first matmul of the group must span all 128 partitions with
            # start=True (start zeroes the whole bank region); the two
            # half-partition identity matmuls then accumulate.
            nc.tensor.matmul(pa, yc[p][:, 0:2, 0, :], a1[:, 0:2, :],
                             start=True, stop=False, perf_mode=DR)
            nc.tensor.matmul(pa[0:U, :], kc0, xT[b0][:, sl],
                             start=False, stop=False)
            nc.tensor.matmul(pa[U:2 * U, :], kc0, xT[b1][:, sl],
                             start=False, stop=False)
            for m, asl in ((0, a1), (1, a2)):
                for jp in range(NJP):
                    if m == 0 and jp == 0:
                        continue
                    js = slice(2 * jp, 2 * jp + 2)
                    nc.tensor.matmul(pa, yc[p][:, js, m, :], asl[:, js, :],
                                     start=False,
                                     stop=(m == 1 and jp == NJP - 1),
                                     perf_mode=DR)
            pas.append(pa)

        if t + 1 < NT:
            a2t2[t + 1] = a2p.tile([NBW, NB, NTW], FP8, name=f"a2b_{t+1}",
                                   tag="a2")
            nc.sync.dma_start(out=a2t2[t + 1], in_=d["a2q"][t + 1])

        for p in range(BL // 2):
            b0, b1 = 2 * p, 2 * p + 1
            pb = psb.tile([NBW, NTW], F32, name="pb2", tag="pb")
            for jp in range(NJP):
                js = slice(2 * jp, 2 * jp + 2)
                nc.tensor.matmul(pb, yc[p][:, js, 2, :], et[t][:, js, :],
                                 start=(jp == 0), stop=(jp == NJP - 1),
                                 perf_mode=DR)
            tmp = stage.tile([NBW, NTW], BF16, name="tmp2", tag="tmp")
            nc.vector.tensor_mul(tmp, pb, rdbc[t])
            ssum = stage.tile([NBW, NTW], BF16, name="ssum2", tag="ssum")
            nc.vector.tensor_add(ssum, pas[p], tmp)
            ct = stage.tile([NBW, NTW], BF16, name="ct", tag="ct")
            t1 = p3p.tile([NBW, NTW], BF16, name="t1", tag="t1")
            # column halves so ACT/DVE/DMA pipeline in the tail;
            # hp is host-pre-halved: h_new = (hp' - ct/2)*tu + (hp' + ct/2)
            for c0 in range(0, NTW, NTW // 2):
                cs = slice(c0, c0 + NTW // 2)
                nc.scalar.activation(ct[:, cs], ssum[:, cs], AF.Tanh,
                                     scale=GSCALE, bias=bc2)
                d2 = stage.tile([NBW, NTW // 2], BF16, name="d2", tag="d2")
                nc.vector.scalar_tensor_tensor(
                    d2, ct[:, cs], -0.5, hps[p][:, cs],
                    op0=OP.mult, op1=OP.add)
                nc.vector.tensor_mul(d2, d2, ut[p][:, sl][:, cs])
                s2 = stage.tile([NBW, NTW // 2], BF16, name="s2", tag="s2")
                nc.vector.scalar_tensor_tensor(
                    s2, ct[:, cs], 0.5, hps[p][:, cs],
                    op0=OP.mult, op1=OP.add)
                nc.vector.tensor_add(t1[:, cs], d2, s2)
                for half, b in ((0, b0), (1, b1)):
                    psl = slice(half * U, (half + 1) * U)
                    nc.sync.dma_start(
                        out=out_h[b][:, t * NTW + c0:t * NTW + c0 + NTW // 2],
                        in_=t1[psl, cs])

    ctx.close()


_CACHE = {}


def _get_program():
    if "nc" not in _CACHE:
        _CACHE["nc"] = _build_program()
    return _CACHE["nc"]


def _prep_inputs(inputs, h_prev, adj1, adj2, feat, SE, Wq, Wk, Ws1, bs1, Ws2,
                 bs2, r_kernel, r_bias, u_kernel, u_bias, c_kernel, c_bias):
    bf = ml_dtypes.bfloat16
    f8 = ml_dtypes.float8_e4m3
    f32 = np.float32
    perm = list(range(DIN, FROWS)) + list(range(DIN))  # [h(64); inputs(2)]

    h3 = np.asarray(h_prev, f32).reshape(B, N, U)
    hT = np.ascontiguousarray(h3.transpose(0, 2, 1))            # [B, U, N]
    inT = np.asarray(inputs, f32).transpose(0, 2, 1)            # [B, DIN, N]
    xT = np.concatenate([hT, inT], axis=1).astype(bf)           # [B, 66, N]
    hTb = (0.5 * hT).astype(bf)   # tanh-form gate combine absorbs the 1/2

    rk = np.asarray(r_kernel, f32)[:, perm, :]
    uk = np.asarray(u_kernel, f32)[:, perm, :]
    ck = np.asarray(c_kernel, f32)[:, perm, :]
    kkall = (BETA * np.concatenate(
        [np.concatenate([rk[m], uk[m]], axis=1) for m in (1, 2, 3)],
        axis=1)).astype(bf)                                     # [66, 384]
    kk0 = (ALPHA * BETA * np.concatenate([rk[0], uk[0]], axis=1)).astype(bf)
    # xT h-rows hold (1+tanh)*h = 2*r*h after phase 1; halve the c-kernel
    # h-rows to compensate
    ck = ck.copy()
    ck[:, 0:U, :] *= 0.5
    kcall = (BETA * np.concatenate([ck[1], ck[2], ck[3]], axis=1)).astype(bf)
    kc0 = (ALPHA * BETA * ck[0]).astype(bf)

    def adj_tiles(a):
        aT = np.asarray(a, f32).T * ALPHA
        np.clip(aT, -240.0, 240.0, out=aT)
        q = aT.astype(f8)                                      # [n, m] = A^T
        # tile[t, p, j, w] = A^T[j*128 + p, t*512 + w]
        return np.ascontiguousarray(
            q.reshape(NB, NBW, NT, NTW).transpose(2, 1, 0, 3))

    shared = {
        "a1q": adj_tiles(adj1),
        "a2q": adj_tiles(adj2),
        "fsT": np.ascontiguousarray(
            np.concatenate([np.asarray(feat, f32).T, np.asarray(SE, f32).T],
                           axis=0)).astype(bf),
        "wq": np.asarray(Wq, f32).astype(bf),
        "wk": np.asarray(Wk, f32).astype(bf),
        "ws1": np.asarray(Ws1, f32).astype(bf),
        "bs1v": np.asarray(bs1, f32).reshape(U, 1),
        "ws2a": (ALPHA * np.asarray(Ws2, f32)).reshape(U, 1).astype(bf),
        "bs2a": (ALPHA * np.asarray(bs2, f32)).reshape(1, 1),
        "kkall": kkall,
        "kk0": kk0,
        "kcall": kcall,
        "kc0": kc0,
        "bru2": 0.5 * np.concatenate(
            [np.asarray(r_bias, f32).mean(0),
             np.asarray(u_bias, f32).mean(0)]).reshape(-1, 1),
        "bc2": np.tile(np.asarray(c_bias, f32).mean(0), 2).reshape(-1, 1),
    }
    in_maps = []
    for c in range(NCORES):
        bsl = slice(c * BL, (c + 1) * BL)
        m = dict(shared)
        m["xT"] = np.ascontiguousarray(xT[bsl])
        m["hT"] = np.ascontiguousarray(hTb[bsl])
        in_maps.append(m)
    return in_maps


def kernel(**inputs):
    os.environ.setdefault("NEURON_RT_RESET_CORES", "1")
    nc = _get_program()
    in_maps = _prep_inputs(**inputs)
    res = None
    err = None
    for _ in range(2):
        try:
            res = run_bass_kernel_spmd(nc, in_maps, list(range(NCORES)))
            break
        except Exception as e:  # e.g. a wedged device; retry once
            err = e
    if res is None:
        raise err
    outs = []
    for c in range(NCORES):
        o = np.asarray(res.results[c]["out"], dtype=np.float32)
        outs.append(o.transpose(0, 2, 1).reshape(BL, N * U))
    return np.concatenate(outs, axis=0).astype(np.float32)


# revision 19
# speedup vs baseline: 2.0942x; 1.0708x over previous
"""MFGCGRU (graph-conv GRU cell) Trainium2 kernel — fp8 DoubleRow edition.

Strategy: data-parallel over batch B=32 across 8 NeuronCores (4 per core).
The diffusion conv is computed kernel-first (S @ (X k)) with the node
contraction run as fp8e4 DoubleRow matmuls (2 node-blocks of 128 per PE
instruction at 0.5 cycles/row): stationary Y-pair [128, 2, 128] fp8,
moving support-pair [128, 2, 512] fp8 (1024-wide moving).

Precision plan (rel-err ~1.3e-2 < 2e-2 on HW):
  - x activations, GRU kernels, identity-support matmul, Y-generation all
    bf16; only the big node-contraction operands (supports + Y) are fp8.
  - supports scaled by ALPHA=64 (adj on host; e via exp ln-bias; the
    sentinel row via host-scaled Ws2/bs2), Y scaled by BETA=16 (host-scaled
    kernels); identity kernels carry ALPHA*BETA; the gate activations
    descale by 0.25/(ALPHA*BETA).

Scheduling: the whole program uses only the `exp_and_others` activation
table — both GRU sigmoids are computed as (1+tanh(z/2))/2 with the /2 and
+1 folded into host-side constants — so the ACT engine never reloads its
function table.  e^T generation for tile t+1 is interleaved into tile t's
A-group matmuls so the PE never waits on the ACT exp evacuations; et tiles
(fp8) and the a1 support slices are cached in SBUF across both phases;
u (as tanh) is kept in SBUF instead of a DRAM round-trip.
"""

import contextlib
import os

import numpy as np
import ml_dtypes

import concourse.bass as bass
import concourse.bacc as bacc
import concourse.tile as tile
from concourse import mybir
from concourse.bass_utils import run_bass_kernel_spmd

F32 = mybir.dt.float32
BF16 = mybir.dt.bfloat16
FP8 = mybir.dt.float8e4
AF = mybir.ActivationFunctionType
DR = mybir.MatmulPerfMode.DoubleRow
OP = mybir.AluOpType

B, N, DIN, U, FD, SD = 32, 2048, 2, 64, 32, 64
NCORES = 8
BL = B // NCORES          # batches per core
NTW = 512                 # n-tile width (output columns per tile)
NT = N // NTW             # 4 n-tiles
NBW = 128                 # node-block width
NB = N // NBW             # 16 node blocks
NJP = NB // 2             # 8 node-block pairs (DoubleRow)
FROWS = DIN + U           # 66

ALPHA = 64.0              # support scale
BETA = 16.0               # Y scale
GSCALE = 0.25 / (ALPHA * BETA)   # gate pre-activation descale (incl /M)
GS2 = GSCALE / 2.0               # tanh-form sigmoid input scale


def _build_program():
    nc = bacc.Bacc("TRN2", debug=False, num_devices=NCORES)

    d = {}

    def din(name, shape, dt):
        d[name] = nc.dram_tensor(name, shape, dt, kind="ExternalInput").ap()

    din("xT", [BL, FROWS, N], BF16)
    din("hT", [BL, U, N], BF16)        # pre-scaled by 0.5 on host
    din("a1q", [NT, NBW, NB, NTW], FP8)
    din("a2q", [NT, NBW, NB, NTW], FP8)
    din("fsT", [FD + SD, N], BF16)
    din("wq", [FD, U], BF16)
    din("wk", [FD, U], BF16)
    din("ws1", [FD + SD, U], BF16)
    din("bs1v", [U, 1], F32)
    din("ws2a", [U, 1], BF16)
    din("bs2a", [1, 1], F32)
    din("kkall", [FROWS, 3 * 2 * U], BF16)
    din("kk0", [FROWS, 2 * U], BF16)
    din("kcall", [FROWS, 3 * U], BF16)
    din("kc0", [FROWS, U], BF16)
    din("bru2", [2 * U, 1], F32)
    din("bc2", [2 * U, 1], F32)
    out_h = nc.dram_tensor("out", [BL, U, N], BF16, kind="ExternalOutput").ap()

    with tile.TileContext(nc) as tc:
        _emit(tc, d, out_h)
    nc.compile()
    return nc


def _interleave(main, extra, ratio=4):
    """Emit `ratio` thunks from main per one from extra."""
    mi = ei = 0
    while mi < len(main) or ei < len(extra):
        for _ in range(ratio):
            if mi < len(main):
                main[mi]()
                mi += 1
        if ei < len(extra):
            extra[ei]()
            ei += 1


def _emit(tc, d, out_h):
    nc = tc.nc
    ctx = contextlib.ExitStack()
    const = ctx.enter_context(tc.tile_pool(name="const", bufs=1))
    persist = ctx.enter_context(tc.tile_pool(name="persist", bufs=1))
    stage = ctx.enter_context(tc.tile_pool(name="stage", bufs=2))
    p3p = ctx.enter_context(tc.tile_pool(name="p3p", bufs=2))
    psacc = ctx.enter_context(tc.tile_pool(name="psacc", bufs=4, space="PSUM"))
    psscr = ctx.enter_context(tc.tile_pool(name="psscr", bufs=2, space="PSUM"))
    psb = ctx.enter_context(tc.tile_pool(name="psb", bufs=2, space="PSUM"))

    # ---- constants / weights in SBUF ----
    def cload(name):
        ap = d[name]
        t = const.tile(list(ap.shape), ap.dtype, name=f"c_{name}")
        nc.sync.dma_start(out=t, in_=ap)
        return t

    fsT = cload("fsT")
    wq = cload("wq")
    wk = cload("wk")
    ws1 = cload("ws1")
    bs1v = cload("bs1v")
    ws2a = cload("ws2a")
    bs2a = cload("bs2a")
    kkall = cload("kkall")
    kk0 = cload("kk0")
    kcall = cload("kcall")
    kc0 = cload("kc0")
    bru2 = cload("bru2")
    bc2 = cload("bc2")

    ones2 = const.tile([NBW, 2, 16], FP8, name="ones2")
    nc.vector.memset(ones2, 1.0)
    ones_row = const.tile([1, NBW], BF16, name="ones_row")
    nc.vector.memset(ones_row, ALPHA)
    lnal = const.tile([NBW, 1], F32, name="lnal")
    nc.vector.memset(lnal, float(np.log(ALPHA)))

    # ---- persistent activations ----
    xT = [persist.tile([FROWS, N], BF16, name=f"xT{b}", tag=f"xT{b}")
          for b in range(BL)]
    nc.sync.dma_start(out=xT[0], in_=d["xT"][0])

    QT = persist.tile([FD, 2, N], FP8, name="QT", tag="QT")
    KT = persist.tile([FD, 2, N], FP8, name="KT", tag="KT")
    s_row = persist.tile([1, N], F32, name="s_row", tag="s_row")
    rdbc = [persist.tile([NBW, NTW], BF16, name=f"rdbc{t}", tag=f"rdbc{t}")
            for t in range(NT)]
    et = [persist.tile([NBW, NB, NTW], FP8, name=f"et{t}", tag=f"et{t}")
          for t in range(NT)]
    a1c = [persist.tile([NBW, NB, NTW], FP8, name=f"a1c{t}", tag=f"a1c{t}")
           for t in range(NT)]
    a2c = [persist.tile([NBW, NB, NTW], FP8, name=f"a2c{t}", tag=f"a2c{t}")
           for t in range(NT)]
    ut = [persist.tile([NBW, N], BF16, name=f"ut{p}", tag=f"ut{p}")
          for p in range(BL // 2)]
    y = [persist.tile([NBW, NB, 3, 2 * U], FP8, name=f"y_{b}", tag=f"y{b}")
         for b in range(BL)]
    yc = [persist.tile([NBW, NB, 3, 2 * U], FP8, name=f"yc_{p}", tag=f"yc{p}")
          for p in range(BL // 2)]

    # adjacency DMAs for phase-1 t=0 (a1 cached for both phases; a2
    # streamed); xT[0] already queued so ygen(b0) can start, adjacency
    # next so A(t0) isn't DMA-starved, remaining xT after
    nc.sync.dma_start(out=a1c[0], in_=d["a1q"][0])
    nc.sync.dma_start(out=a2c[0], in_=d["a2q"][0])
    for b in range(1, BL):
        nc.sync.dma_start(out=xT[b], in_=d["xT"][b])

    # ---- thunk generators ----
    def prelude_thunks():
        th = []
        for t in range(NT):
            sl = slice(t * NTW, (t + 1) * NTW)
            # KT before QT: the interleaved e-gen(t0) thunk ei fires after
            # main[4*ei+3]; ei=0 needs all four K/Q writes of the t0 block
            # already emitted (Tile cannot depend on future instructions)
            for dst, w in ((KT, wk), (QT, wq)):
                for uh in range(2):
                    def f(dst=dst, w=w, uh=uh, sl=sl):
                        pq = psacc.tile([FD, NTW], F32, name="pq", tag="acc")
                        nc.tensor.matmul(pq, w[:, uh * FD:(uh + 1) * FD],
                                         fsT[0:FD, sl], start=True, stop=True)
                        if uh == 0:
                            nc.scalar.activation(dst[:, uh, sl], pq, AF.Relu)
                        else:
                            nc.vector.tensor_scalar_max(dst[:, uh, sl], pq, 0.0)
                    th.append(f)

            def g(sl=sl):
                ps1 = psacc.tile([U, NTW], F32, name="ps1", tag="acc")
                nc.tensor.matmul(ps1, ws1, fsT[:, sl], start=True, stop=True)
                s1t = stage.tile([U, NTW], BF16, name="s1t", tag="s1t")
                nc.scalar.activation(s1t, ps1, AF.Relu, bias=bs1v)
                ps2 = psacc.tile([1, NTW], F32, name="ps2", tag="acc")
                nc.tensor.matmul(ps2, ws2a, s1t, start=True, stop=True)
                nc.scalar.activation(s_row[:, sl], ps2, AF.Relu, bias=bs2a)
            th.append(g)
        return th

    def ygen_thunks(b):
        th = []
        for j in range(NB):
            def f(b=b, j=j):
                nsl = slice(j * NBW, (j + 1) * NBW)
                py = psscr.tile([NBW, 3 * 2 * U], F32, name="py", tag="scr")
                nc.tensor.matmul(py, xT[b][:, nsl], kkall,
                                 start=True, stop=True)
                src = py.rearrange("p (m u) -> p m u", m=3)
                if (b * NB + j) % 16 < 11:
                    nc.vector.tensor_copy(y[b][:, j, :, :], src)
                else:
                    nc.scalar.activation(y[b][:, j, :, :], src, AF.Copy)
            th.append(f)
        return th

    def ycgen_thunks(p):
        th = []
        for half in range(2):
            b = 2 * p + half
            usl = slice(half * U, (half + 1) * U)
            for j0 in range(0, NB, 2):
                def f(b=b, usl=usl, j0=j0, p=p):
                    pyc = psscr.tile([NBW, 2 * 3 * U], F32, name="pyc",
                                     tag="scr")
                    for i in range(2):
                        nsl = slice((j0 + i) * NBW, (j0 + i + 1) * NBW)
                        # start zeroes the whole PSUM bank region: only the
                        # first matmul of the pair may set it
                        nc.tensor.matmul(pyc[:, i * 3 * U:(i + 1) * 3 * U],
                                         xT[b][:, nsl], kcall,
                                         start=(i == 0), stop=(i == 1))
                    src = pyc.rearrange("p (j m u) -> p j m u", j=2, m=3)
                    if (b * NB + j0) % 4 < 2:
                        nc.vector.tensor_copy(yc[p][:, j0:j0 + 2, :, usl], src)
                    else:
                        nc.scalar.activation(yc[p][:, j0:j0 + 2, :, usl], src,
                                             AF.Copy)
                th.append(f)
        return th

    def egen_thunks(t):
        sl = slice(t * NTW, (t + 1) * NTW)
        th = []
        for j in range(NB):
            def f(j=j, t=t, sl=sl):
                pe = psscr.tile([NBW, NTW], F32, name="pe", tag="scr")
                nc.tensor.matmul(pe, KT[:, :, j * NBW:(j + 1) * NBW],
                                 QT[:, :, sl], start=True, stop=True,
                                 perf_mode=DR)
                nc.scalar.activation(et[t][:, j, :], pe, AF.Exp, scale=0.125,
                                     bias=lnal)
            th.append(f)
        return th

    def agroup1_thunks(b, t, a1, a2, pa):
        sl = slice(t * NTW, (t + 1) * NTW)
        head = [lambda: nc.tensor.matmul(pa, kk0, xT[b][:, sl],
                                         start=True, stop=False)]
        tail = []
        for m, asl, out in ((0, a1, head), (1, a2, tail)):
            for jp in range(NJP):
                def f(m=m, asl=asl, jp=jp, b=b):
                    js = slice(2 * jp, 2 * jp + 2)
                    nc.tensor.matmul(pa, y[b][:, js, m, :], asl[:, js, :],
                                     start=False,
                                     stop=(m == 1 and jp == NJP - 1),
                                     perf_mode=DR)
                out.append(f)
        return head, tail

    def dblock(t):
        sl = slice(t * NTW, (t + 1) * NTW)
        pd = psb.tile([1, NTW], F32, name="pd", tag="pb")
        for jp in range(NJP):
            js = slice(2 * jp, 2 * jp + 2)
            nc.tensor.matmul(pd, ones2[:, :, 0:1], et[t][:, js, :],
                             start=(jp == 0), stop=(jp == NJP - 1),
                             perf_mode=DR)
        dsb = stage.tile([1, NTW], F32, name="dsb", tag="dsb")
        nc.vector.tensor_add(dsb, pd, s_row[:, sl])
        rds = stage.tile([1, NTW], BF16, name="rds", tag="dsb")
        with nc.allow_low_precision(reason="rdbc is bf16 by design"):
            nc.vector.reciprocal(rds, dsb)
        pr = psb.tile([NBW, NTW], F32, name="pr", tag="pb")
        nc.tensor.matmul(pr, ones_row, rds, start=True, stop=True)
        nc.vector.tensor_copy(rdbc[t], pr)

    def bgroup1(b, t, pa):
        sl = slice(t * NTW, (t + 1) * NTW)
        pb = psb.tile([NBW, NTW], F32, name="pb", tag="pb")
        for jp in range(NJP):
            js = slice(2 * jp, 2 * jp + 2)
            nc.tensor.matmul(pb, y[b][:, js, 2, :], et[t][:, js, :],
                             start=(jp == 0), stop=(jp == NJP - 1),
                             perf_mode=DR)
        tmp = stage.tile([NBW, NTW], BF16, name="tmp", tag="tmp")
        nc.vector.tensor_mul(tmp, pb, rdbc[t])
        ssum = stage.tile([NBW, NTW], BF16, name="ssum", tag="ssum")
        nc.vector.tensor_add(ssum, pa, tmp)
        # th = tanh(GS2*ssum + bru/2): rows 0:64 -> (1+th)*h into xT h-rows
        # (factor 2 absorbed in host-halved kcall/kc0 h-rows); rows 64:128
        # -> ut stores tanh-form u
        th = stage.tile([NBW, NTW], BF16, name="th", tag="th")
        nc.scalar.activation(th, ssum, AF.Tanh, scale=GS2, bias=bru2)
        nc.vector.scalar_tensor_tensor(
            xT[b][0:U, sl], th[0:U, :], 1.0, xT[b][0:U, sl],
            op0=OP.add, op1=OP.mult)
        p, half = b // 2, b % 2
        nc.vector.tensor_copy(ut[p][half * U:(half + 1) * U, sl],
                              th[U:2 * U, :])

    # ==================== pre-phase ====================
    pre = prelude_thunks() + ygen_thunks(0) + ygen_thunks(1)
    _interleave(pre, egen_thunks(0), ratio=6)

    # ==================== phase 1: r & u gates ====================
    for t in range(NT):
        sl = slice(t * NTW, (t + 1) * NTW)
        a1, a2 = a1c[t], a2c[t]

        pas = []
        heads, tails = [], []
        for b in range(BL):
            pa = psacc.tile([NBW, NTW], F32, name="pa", tag="acc")
            hd, tl = agroup1_thunks(b, t, a1, a2, pa)
            heads.extend(hd)
            tails.extend(tl)
            pas.append(pa)
            if t == 0 and b == 0:
                heads.extend(ygen_thunks(2))
            if t == 0 and b == 1:
                heads.extend(ygen_thunks(3))
        # id + a1 contributions for every batch first: at t=0 the a2 slice
        # is still streaming in while these run
        _interleave(heads + tails,
                    egen_thunks(t + 1) if t + 1 < NT else [], ratio=4)

        if t + 1 < NT:
            nc.sync.dma_start(out=a1c[t + 1], in_=d["a1q"][t + 1])
            nc.sync.dma_start(out=a2c[t + 1], in_=d["a2q"][t + 1])

        if t == 0:
            dblock(0)
        for b in range(BL):
            bgroup1(b, t, pas[b])
        # next tile's normalizer now: its exps completed during this tile's
        # A-groups, and emitting it here keeps rdbc(t+1) clear of the
        # B-epilogue critical path
        if t + 1 < NT:
            dblock(t + 1)

    # ==================== phase 2: c gate & h_new ====================
    # node-blocks j<12 depend only on t0..t2 epilogues — emit them first so
    # the PE keeps busy while the t3 epilogue chain drains; yc[1]'s late
    # blocks are interleaved into phase-2 t0's first A-group
    yc0, yc1 = ycgen_thunks(0), ycgen_thunks(1)
    for f in [f for i, f in enumerate(yc0) if (i % 8) < 6]:
        f()
    for f in [f for i, f in enumerate(yc0) if (i % 8) >= 6]:
        f()
    for f in [f for i, f in enumerate(yc1) if (i % 8) < 6]:
        f()
    yc1_rest = [f for i, f in enumerate(yc1) if (i % 8) >= 6]

    for t in range(NT):
        sl = slice(t * NTW, (t + 1) * NTW)
        a1, a2 = a1c[t], a2c[t]

        hps = []
        for p in range(BL // 2):
            hp = p3p.tile([NBW, NTW], BF16, name="hp", tag="hp")
            nc.sync.dma_start(
                out=hp, in_=d["hT"][2 * p:2 * p + 2, :, sl])
            hps.append(hp)

        pas = []
        for p in range(BL // 2):
            b0, b1 = 2 * p, 2 * p + 1
            pa = psacc.tile([NBW, NTW], F32, name="pa2", tag="acc")
            # first matmul of the group must span all 128 partitions with
            # start=True (start zeroes the whole bank region); the two
            # half-partition identity matmuls then accumulate.
            amain = [lambda: nc.tensor.matmul(
                         pa, yc[p][:, 0:2, 0, :], a1[:, 0:2, :],
                         start=True, stop=False, perf_mode=DR),
                     lambda: nc.tensor.matmul(
                         pa[0:U, :], kc0, xT[b0][:, sl],
                         start=False, stop=False),
                     lambda: nc.tensor.matmul(
                         pa[U:2 * U, :], kc0, xT[b1][:, sl],
                         start=False, stop=False)]
            for m, asl in ((0, a1), (1, a2)):
                for jp in range(NJP):
                    if m == 0 and jp == 0:
                        continue
                    def f(m=m, asl=asl, jp=jp, p=p, pa=pa):
                        js = slice(2 * jp, 2 * jp + 2)
                        nc.tensor.matmul(pa, yc[p][:, js, m, :],
                                         asl[:, js, :], start=False,
                                         stop=(m == 1 and jp == NJP - 1),
                                         perf_mode=DR)
                    amain.append(f)
            extra = yc1_rest if (t == 0 and p == 0) else []
            _interleave(amain, extra, ratio=3)
            pas.append(pa)

            pb = psb.tile([NBW, NTW], F32, name="pb2", tag="pb")
            for jp in range(NJP):
                js = slice(2 * jp, 2 * jp + 2)
                nc.tensor.matmul(pb, yc[p][:, js, 2, :], et[t][:, js, :],
                                 start=(jp == 0), stop=(jp == NJP - 1),
                                 perf_mode=DR)
            tmp = stage.tile([NBW, NTW], BF16, name="tmp2", tag="tmp")
            nc.vector.tensor_mul(tmp, pb, rdbc[t])
            ssum = stage.tile([NBW, NTW], BF16, name="ssum2", tag="ssum")
            nc.vector.tensor_add(ssum, pas[p], tmp)
            ct = stage.tile([NBW, NTW], BF16, name="ct", tag="ct")
            t1 = p3p.tile([NBW, NTW], BF16, name="t1", tag="t1")
            # column halves so ACT/DVE/DMA pipeline in the tail;
            # hp is host-pre-halved: h_new = (hp' - ct/2)*tu + (hp' + ct/2)
            for c0 in range(0, NTW, NTW // 2):
                cs = slice(c0, c0 + NTW // 2)
                nc.scalar.activation(ct[:, cs], ssum[:, cs], AF.Tanh,
                                     scale=GSCALE, bias=bc2)
                d2 = stage.tile([NBW, NTW // 2], BF16, name="d2", tag="d2")
                nc.vector.scalar_tensor_tensor(
                    d2, ct[:, cs], -0.5, hps[p][:, cs],
                    op0=OP.mult, op1=OP.add)
                nc.vector.tensor_mul(d2, d2, ut[p][:, sl][:, cs])
                s2 = stage.tile([NBW, NTW // 2], BF16, name="s2", tag="s2")
                nc.vector.scalar_tensor_tensor(
                    s2, ct[:, cs], 0.5, hps[p][:, cs],
                    op0=OP.mult, op1=OP.add)
                nc.vector.tensor_add(t1[:, cs], d2, s2)
                c1 = t * NTW + c0
                nc.sync.dma_start(
                    out=out_h[2 * p:2 * p + 2, :, c1:c1 + NTW // 2],
                    in_=t1[:, cs])

    ctx.close()


_CACHE = {}


def _get_program():
    if "nc" not in _CACHE:
        _CACHE["nc"] = _build_program()
    return _CACHE["nc"]


def _prep_inputs(inputs, h_prev, adj1, adj2, feat, SE, Wq, Wk, Ws1, bs1, Ws2,
                 bs2, r_kernel, r_bias, u_kernel, u_bias, c_kernel, c_bias):
    bf = ml_dtypes.bfloat16
    f8 = ml_dtypes.float8_e4m3
    f32 = np.float32
    perm = list(range(DIN, FROWS)) + list(range(DIN))  # [h(64); inputs(2)]

    h3 = np.asarray(h_prev, f32).reshape(B, N, U)
    hT = np.ascontiguousarray(h3.transpose(0, 2, 1))            # [B, U, N]
    inT = np.asarray(inputs, f32).transpose(0, 2, 1)            # [B, DIN, N]
    xT = np.concatenate([hT, inT], axis=1).astype(bf)           # [B, 66, N]
    hTb = (0.5 * hT).astype(bf)   # tanh-form gate combine absorbs the 1/2

    rk = np.asarray(r_kernel, f32)[:, perm, :]
    uk = np.asarray(u_kernel, f32)[:, perm, :]
    ck = np.asarray(c_kernel, f32)[:, perm, :]
    kkall = (BETA * np.concatenate(
        [np.concatenate([rk[m], uk[m]], axis=1) for m in (1, 2, 3)],
        axis=1)).astype(bf)                                     # [66, 384]
    kk0 = (ALPHA * BETA * np.concatenate([rk[0], uk[0]], axis=1)).astype(bf)
    # xT h-rows hold (1+tanh)*h = 2*r*h after phase 1; halve the c-kernel
    # h-rows to compensate
    ck = ck.copy()
    ck[:, 0:U, :] *= 0.5
    kcall = (BETA * np.concatenate([ck[1], ck[2], ck[3]], axis=1)).astype(bf)
    kc0 = (ALPHA * BETA * ck[0]).astype(bf)

    def adj_tiles(a):
        aT = np.asarray(a, f32).T * ALPHA
        np.clip(aT, -240.0, 240.0, out=aT)
        q = aT.astype(f8)                                      # [n, m] = A^T
        # tile[t, p, j, w] = A^T[j*128 + p, t*512 + w]
        return np.ascontiguousarray(
            q.reshape(NB, NBW, NT, NTW).transpose(2, 1, 0, 3))

    shared = {
        "a1q": adj_tiles(adj1),
        "a2q": adj_tiles(adj2),
        "fsT": np.ascontiguousarray(
            np.concatenate([np.asarray(feat, f32).T, np.asarray(SE, f32).T],
                           axis=0)).astype(bf),
        "wq": np.asarray(Wq, f32).astype(bf),
        "wk": np.asarray(Wk, f32).astype(bf),
        "ws1": np.asarray(Ws1, f32).astype(bf),
        "bs1v": np.asarray(bs1, f32).reshape(U, 1),
        "ws2a": (ALPHA * np.asarray(Ws2, f32)).reshape(U, 1).astype(bf),
        "bs2a": (ALPHA * np.asarray(bs2, f32)).reshape(1, 1),
        "kkall": kkall,
        "kk0": kk0,
        "kcall": kcall,
        "kc0": kc0,
        "bru2": 0.5 * np.concatenate(
            [np.asarray(r_bias, f32).mean(0),
             np.asarray(u_bias, f32).mean(0)]).reshape(-1, 1),
        "bc2": np.tile(np.asarray(c_bias, f32).mean(0), 2).reshape(-1, 1),
    }
    in_maps = []
    for c in range(NCORES):
        bsl = slice(c * BL, (c + 1) * BL)
        m = dict(shared)
        m["xT"] = np.ascontiguousarray(xT[bsl])
        m["hT"] = np.ascontiguousarray(hTb[bsl])
        in_maps.append(m)
    return in_maps


def kernel(**inputs):
    os.environ.setdefault("NEURON_RT_RESET_CORES", "1")
    nc = _get_program()
    in_maps = _prep_inputs(**inputs)
    res = None
    err = None
    for _ in range(2):
        try:
            res = run_bass_kernel_spmd(nc, in_maps, list(range(NCORES)))
            break
        except Exception as e:  # e.g. a wedged device; retry once
            err = e
    if res is None:
        raise err
    outs = []
    for c in range(NCORES):
        o = np.asarray(res.results[c]["out"], dtype=np.float32)
        outs.append(o.transpose(0, 2, 1).reshape(BL, N * U))
    return np.concatenate(outs, axis=0).astype(np.float32)


# revision 28
# speedup vs baseline: 2.3417x; 1.1182x over previous
"""MFGCGRU (graph-conv GRU cell) Trainium2 kernel — fp8 DoubleRow edition.

Strategy: data-parallel over batch B=32 across 8 NeuronCores (4 per core).
The diffusion conv is computed kernel-first (S @ (X k)) with the node
contraction run as fp8e4 DoubleRow matmuls (2 node-blocks of 128 per PE
instruction at 0.5 cycles/row): stationary Y-pair [128, 2, 128] fp8,
moving support-pair [128, 2, 512] fp8 (1024-wide moving).

Precision plan (rel-err ~1.3e-2 < 2e-2 on HW):
  - x activations, GRU kernels, identity-support matmul, Y-generation all
    bf16; only the big node-contraction operands (supports + Y) are fp8.
  - supports scaled by ALPHA=64 (adj on host; e via exp ln-bias; the
    sentinel row via host-scaled Ws2/bs2), Y scaled by BETA=16 (host-scaled
    kernels); identity kernels carry ALPHA*BETA; the gate activations
    descale by 0.25/(ALPHA*BETA).

Scheduling: the whole program uses only the `exp_and_others` activation
table — both GRU sigmoids are computed as (1+tanh(z/2))/2 with the /2 and
+1 folded into host-side constants — so the ACT engine never reloads its
function table.  e^T generation for tile t+1 is interleaved into tile t's
A-group matmuls so the PE never waits on the ACT exp evacuations; et tiles
(fp8) and the a1 support slices are cached in SBUF across both phases;
u (as tanh) is kept in SBUF instead of a DRAM round-trip.
"""

import contextlib
import os

import numpy as np
import ml_dtypes

import concourse.bass as bass
import concourse.bacc as bacc
import concourse.tile as tile
from concourse import mybir
from concourse.bass_utils import run_bass_kernel_spmd

F32 = mybir.dt.float32
BF16 = mybir.dt.bfloat16
FP8 = mybir.dt.float8e4
AF = mybir.ActivationFunctionType
DR = mybir.MatmulPerfMode.DoubleRow
OP = mybir.AluOpType

B, N, DIN, U, FD, SD = 32, 2048, 2, 64, 32, 64
NCORES = 8
BL = B // NCORES          # batches per core
NTW = 512                 # n-tile width (output columns per tile)
NT = N // NTW             # 4 n-tiles
NBW = 128                 # node-block width
NB = N // NBW             # 16 node blocks
NJP = NB // 2             # 8 node-block pairs (DoubleRow)
FROWS = DIN + U           # 66

ALPHA = 64.0              # support scale
BETA = 16.0               # Y scale
GSCALE = 0.25 / (ALPHA * BETA)   # gate pre-activation descale (incl /M)
GS2 = GSCALE / 2.0               # tanh-form sigmoid input scale


def _build_program():
    nc = bacc.Bacc("TRN2", debug=False, num_devices=NCORES)

    d = {}

    def din(name, shape, dt):
        d[name] = nc.dram_tensor(name, shape, dt, kind="ExternalInput").ap()

    din("xT", [BL, FROWS, N], BF16)
    din("hT", [BL, U, N], BF16)        # pre-scaled by 0.5 on host
    din("a1q", [NT, NBW, NB, NTW], FP8)
    din("a2q", [NT, NBW, NB, NTW], FP8)
    din("fsT", [FD + SD, N], BF16)
    din("wq", [FD, U], BF16)
    din("wk", [FD, U], BF16)
    din("ws1", [FD + SD, U], BF16)
    din("bs1v", [U, 1], F32)
    din("ws2a", [U, 1], BF16)
    din("bs2a", [1, 1], F32)
    din("kkall", [FROWS, 3 * 2 * U], BF16)
    din("kk0", [FROWS, 2 * U], BF16)
    din("kcall", [FROWS, 3 * U], BF16)
    din("kc0", [FROWS, U], BF16)
    din("bru2", [2 * U, 1], F32)
    din("bc2", [2 * U, 1], F32)
    out_h = nc.dram_tensor("out", [BL, U, N], BF16, kind="ExternalOutput").ap()

    with tile.TileContext(nc) as tc:
        _emit(tc, d, out_h)
    nc.compile()
    return nc


def _interleave(main, extra, ratio=4):
    """Emit `ratio` thunks from main per one from extra."""
    mi = ei = 0
    while mi < len(main) or ei < len(extra):
        for _ in range(ratio):
            if mi < len(main):
                main[mi]()
                mi += 1
        if ei < len(extra):
            extra[ei]()
            ei += 1


def _emit(tc, d, out_h):
    nc = tc.nc
    ctx = contextlib.ExitStack()
    const = ctx.enter_context(tc.tile_pool(name="const", bufs=1))
    persist = ctx.enter_context(tc.tile_pool(name="persist", bufs=1))
    stage = ctx.enter_context(tc.tile_pool(name="stage", bufs=2))
    p3p = ctx.enter_context(tc.tile_pool(name="p3p", bufs=2))
    psacc = ctx.enter_context(tc.tile_pool(name="psacc", bufs=4, space="PSUM"))
    psscr = ctx.enter_context(tc.tile_pool(name="psscr", bufs=2, space="PSUM"))
    psb = ctx.enter_context(tc.tile_pool(name="psb", bufs=2, space="PSUM"))

    # ---- constants / weights in SBUF ----
    def cload(name):
        ap = d[name]
        t = const.tile(list(ap.shape), ap.dtype, name=f"c_{name}")
        nc.sync.dma_start(out=t, in_=ap)
        return t

    fsT = cload("fsT")
    wq = cload("wq")
    wk = cload("wk")
    ws1 = cload("ws1")
    bs1v = cload("bs1v")
    ws2a = cload("ws2a")
    bs2a = cload("bs2a")
    kkall = cload("kkall")
    kk0 = cload("kk0")
    kcall = cload("kcall")
    kc0 = cload("kc0")
    bru2 = cload("bru2")
    bc2 = cload("bc2")

    ones2 = const.tile([NBW, 2, 16], FP8, name="ones2")
    nc.vector.memset(ones2, 1.0)
    ones_row = const.tile([1, NBW], BF16, name="ones_row")
    nc.vector.memset(ones_row, ALPHA)
    lnal = const.tile([NBW, 1], F32, name="lnal")
    nc.vector.memset(lnal, float(np.log(ALPHA)))

    # ---- persistent activations ----
    xT = [persist.tile([FROWS, N], BF16, name=f"xT{b}", tag=f"xT{b}")
          for b in range(BL)]
    nc.sync.dma_start(out=xT[0], in_=d["xT"][0])

    QT = persist.tile([FD, 2, N], FP8, name="QT", tag="QT")
    KT = persist.tile([FD, 2, N], FP8, name="KT", tag="KT")
    s_row = persist.tile([1, N], BF16, name="s_row", tag="s_row")
    rdbc = [persist.tile([NBW, NTW], BF16, name=f"rdbc{t}", tag=f"rdbc{t}")
            for t in range(NT)]
    et = [persist.tile([NBW, NB, NTW], FP8, name=f"et{t}", tag=f"et{t}")
          for t in range(NT)]
    a1c = [persist.tile([NBW, NB, NTW], FP8, name=f"a1c{t}", tag=f"a1c{t}")
           for t in range(NT)]
    a2c = [persist.tile([NBW, NB, NTW], FP8, name=f"a2c{t}", tag=f"a2c{t}")
           for t in range(NT)]
    ut = [persist.tile([NBW, N], BF16, name=f"ut{p}", tag=f"ut{p}")
          for p in range(BL // 2)]
    y = [persist.tile([NBW, NB, 3, 2 * U], FP8, name=f"y_{b}", tag=f"y{b}")
         for b in range(BL)]
    yc = [persist.tile([NBW, NB, 3, 2 * U], FP8, name=f"yc_{p}", tag=f"yc{p}")
          for p in range(BL // 2)]

    # adjacency DMAs for phase-1 t=0 (a1 cached for both phases; a2
    # streamed); xT[0] already queued so ygen(b0) can start, adjacency
    # next so A(t0) isn't DMA-starved, remaining xT after
    nc.sync.dma_start(out=a1c[0], in_=d["a1q"][0])
    nc.sync.dma_start(out=a2c[0], in_=d["a2q"][0])
    for b in range(1, BL):
        nc.sync.dma_start(out=xT[b], in_=d["xT"][b])

    # ---- thunk generators ----
    def prelude_thunks():
        th = []
        for t in range(NT):
            sl = slice(t * NTW, (t + 1) * NTW)
            # KT before QT: the interleaved e-gen(t0) thunk ei fires after
            # main[4*ei+3]; ei=0 needs all four K/Q writes of the t0 block
            # already emitted (Tile cannot depend on future instructions)
            for dst, w in ((KT, wk), (QT, wq)):
                for uh in range(2):
                    def f(dst=dst, w=w, uh=uh, sl=sl):
                        pq = psacc.tile([FD, NTW], F32, name="pq", tag="acc")
                        nc.tensor.matmul(pq, w[:, uh * FD:(uh + 1) * FD],
                                         fsT[0:FD, sl], start=True, stop=True)
                        if uh == 0:
                            nc.scalar.activation(dst[:, uh, sl], pq, AF.Relu)
                        else:
                            nc.vector.tensor_scalar_max(dst[:, uh, sl], pq, 0.0)
                    th.append(f)

            def g(sl=sl):
                ps1 = psacc.tile([U, NTW], F32, name="ps1", tag="acc")
                nc.tensor.matmul(ps1, ws1, fsT[:, sl], start=True, stop=True)
                s1t = stage.tile([U, NTW], BF16, name="s1t", tag="s1t", bufs=1)
                nc.scalar.activation(s1t, ps1, AF.Relu, bias=bs1v)
                ps2 = psacc.tile([1, NTW], F32, name="ps2", tag="acc")
                nc.tensor.matmul(ps2, ws2a, s1t, start=True, stop=True)
                nc.scalar.activation(s_row[:, sl], ps2, AF.Relu, bias=bs2a)
            th.append(g)
        return th

    def ygen_thunks(b):
        th = []
        for j in range(NB):
            def f(b=b, j=j):
                nsl = slice(j * NBW, (j + 1) * NBW)
                py = psscr.tile([NBW, 3 * 2 * U], F32, name="py", tag="scr")
                nc.tensor.matmul(py, xT[b][:, nsl], kkall,
                                 start=True, stop=True)
                src = py.rearrange("p (m u) -> p m u", m=3)
                if (b * NB + j) % 16 < 11:
                    nc.vector.tensor_copy(y[b][:, j, :, :], src)
                else:
                    nc.scalar.activation(y[b][:, j, :, :], src, AF.Copy)
            th.append(f)
        return th

    def ycgen_thunks(p):
        th = []
        for half in range(2):
            b = 2 * p + half
            usl = slice(half * U, (half + 1) * U)
            for j0 in range(0, NB, 2):
                def f(b=b, usl=usl, j0=j0, p=p):
                    pool_ = psscr if (j0 // 2) % 2 == 0 else psb
                    tag_ = "scr" if (j0 // 2) % 2 == 0 else "pb"
                    pyc = pool_.tile([NBW, 2 * 3 * U], F32, name="pyc",
                                     tag=tag_)
                    for i in range(2):
                        nsl = slice((j0 + i) * NBW, (j0 + i + 1) * NBW)
                        # start zeroes the whole PSUM bank region: only the
                        # first matmul of the pair may set it
                        nc.tensor.matmul(pyc[:, i * 3 * U:(i + 1) * 3 * U],
                                         xT[b][:, nsl], kcall,
                                         start=(i == 0), stop=(i == 1))
                    src = pyc.rearrange("p (j m u) -> p j m u", j=2, m=3)
                    if (b * NB + j0) % 4 < 2:
                        nc.vector.tensor_copy(yc[p][:, j0:j0 + 2, :, usl], src)
                    else:
                        nc.scalar.activation(yc[p][:, j0:j0 + 2, :, usl], src,
                                             AF.Copy)
                th.append(f)
        return th

    def egen_thunks(t):
        sl = slice(t * NTW, (t + 1) * NTW)
        th = []
        for j in range(NB):
            def f(j=j, t=t, sl=sl):
                pe = psscr.tile([NBW, NTW], F32, name="pe", tag="scr")
                nc.tensor.matmul(pe, KT[:, :, j * NBW:(j + 1) * NBW],
                                 QT[:, :, sl], start=True, stop=True,
                                 perf_mode=DR)
                nc.scalar.activation(et[t][:, j, :], pe, AF.Exp, scale=0.125,
                                     bias=lnal)
            th.append(f)
        return th

    def agroup1_thunks(b, t, a1, a2, pa):
        sl = slice(t * NTW, (t + 1) * NTW)
        head = [lambda: nc.tensor.matmul(pa, kk0, xT[b][:, sl],
                                         start=True, stop=False)]
        tail = []
        for m, asl, out in ((0, a1, head), (1, a2, tail)):
            for jp in range(NJP):
                def f(m=m, asl=asl, jp=jp, b=b):
                    js = slice(2 * jp, 2 * jp + 2)
                    nc.tensor.matmul(pa, y[b][:, js, m, :], asl[:, js, :],
                                     start=False,
                                     stop=(m == 1 and jp == NJP - 1),
                                     perf_mode=DR)
                out.append(f)
        return head, tail

    def dblock(t):
        sl = slice(t * NTW, (t + 1) * NTW)
        pd = psb.tile([1, NTW], F32, name="pd", tag="pb")
        for jp in range(NJP):
            js = slice(2 * jp, 2 * jp + 2)
            nc.tensor.matmul(pd, ones2[:, :, 0:1], et[t][:, js, :],
                             start=(jp == 0), stop=(jp == NJP - 1),
                             perf_mode=DR)
        dsb = stage.tile([1, NTW], F32, name="dsb", tag="dsb", bufs=1)
        nc.vector.tensor_add(dsb, pd, s_row[:, sl])
        rds = stage.tile([1, NTW], BF16, name="rds", tag="rds", bufs=1)
        with nc.allow_low_precision(reason="rdbc is bf16 by design"):
            nc.vector.reciprocal(rds, dsb)
        pr = psb.tile([NBW, NTW], F32, name="pr", tag="pb")
        nc.tensor.matmul(pr, ones_row, rds, start=True, stop=True)
        nc.vector.tensor_copy(rdbc[t], pr)

    def bgroup1_all(t, pas):
        """B-matmuls then a breadth-first epilogue: each DVE stage for all
        four batches before the next, so the strict-FIFO DVE queue never
        head-blocks on an ACT result while later batches' work is ready.
        pb ring is 2 deep: tmp(b) is emitted right when pb(b+2) needs the
        slot back."""
        sl = slice(t * NTW, (t + 1) * NTW)
        pbs, tmps, ssums, ths = [], [], [], []
        for b in range(BL):
            pb = psb.tile([NBW, NTW], F32, name="pb", tag="pb")
            for jp in range(NJP):
                js = slice(2 * jp, 2 * jp + 2)
                nc.tensor.matmul(pb, y[b][:, js, 2, :], et[t][:, js, :],
                                 start=(jp == 0), stop=(jp == NJP - 1),
                                 perf_mode=DR)
            pbs.append(pb)
            tmp = stage.tile([NBW, NTW], BF16, name="tmp", tag="tmp",
                             bufs=4)
            nc.vector.tensor_mul(tmp, pb, rdbc[t])
            tmps.append(tmp)
        for b in range(BL):
            ssum = stage.tile([NBW, NTW], BF16, name="ssum", tag="ssum",
                              bufs=4)
            nc.vector.tensor_add(ssum, pas[b], tmps[b])
            ssums.append(ssum)
        for b in range(BL):
            # th = tanh(GS2*ssum + bru/2): rows 0:64 -> (1+th)*h into xT
            # h-rows (factor 2 absorbed in host-halved kcall/kc0 h-rows);
            # rows 64:128 -> ut stores tanh-form u
            th = stage.tile([NBW, NTW], BF16, name="th", tag="th", bufs=4)
            nc.scalar.activation(th, ssums[b], AF.Tanh, scale=GS2,
                                 bias=bru2)
            ths.append(th)
        for b in range(BL):
            nc.vector.scalar_tensor_tensor(
                xT[b][0:U, sl], ths[b][0:U, :], 1.0, xT[b][0:U, sl],
                op0=OP.add, op1=OP.mult)
        for b in range(BL):
            p, half = b // 2, b % 2
            nc.vector.tensor_copy(ut[p][half * U:(half + 1) * U, sl],
                                  ths[b][U:2 * U, :])

    # ==================== pre-phase ====================
    pre = prelude_thunks() + ygen_thunks(0) + ygen_thunks(1)
    _interleave(pre, egen_thunks(0), ratio=6)

    # ==================== phase 1: r & u gates ====================
    for t in range(NT):
        sl = slice(t * NTW, (t + 1) * NTW)
        a1, a2 = a1c[t], a2c[t]

        pas = []
        heads, tails = [], []
        for b in range(BL):
            pa = psacc.tile([NBW, NTW], F32, name="pa", tag="acc")
            hd, tl = agroup1_thunks(b, t, a1, a2, pa)
            heads.extend(hd)
            tails.extend(tl)
            pas.append(pa)
            if t == 0 and b == 0:
                heads.extend(ygen_thunks(2))
            if t == 0 and b == 1:
                heads.extend(ygen_thunks(3))
        # id + a1 contributions for every batch first: at t=0 the a2 slice
        # is still streaming in while these run
        _interleave(heads + tails,
                    egen_thunks(t + 1) if t + 1 < NT else [], ratio=4)

        if t + 1 < NT:
            nc.sync.dma_start(out=a1c[t + 1], in_=d["a1q"][t + 1])
            nc.sync.dma_start(out=a2c[t + 1], in_=d["a2q"][t + 1])

        if t == 0:
            dblock(0)
        bgroup1_all(t, pas)
        # next tile's normalizer now: its exps completed during this tile's
        # A-groups, and emitting it here keeps rdbc(t+1) clear of the
        # B-epilogue critical path
        if t + 1 < NT:
            dblock(t + 1)

    # ==================== phase 2: c gate & h_new ====================
    # node-blocks j<12 depend only on t0..t2 epilogues — emit them first so
    # the PE keeps busy while the t3 epilogue chain drains; yc[1]'s late
    # blocks are interleaved into phase-2 t0's first A-group
    yc0, yc1 = ycgen_thunks(0), ycgen_thunks(1)
    for f in [f for i, f in enumerate(yc0) if (i % 8) < 6]:
        f()
    for f in [f for i, f in enumerate(yc0) if (i % 8) >= 6]:
        f()
    for f in [f for i, f in enumerate(yc1) if (i % 8) < 6]:
        f()
    yc1_rest = [f for i, f in enumerate(yc1) if (i % 8) >= 6]

    for t in range(NT):
        sl = slice(t * NTW, (t + 1) * NTW)
        a1, a2 = a1c[t], a2c[t]

        if t == 0:
            hps = []
            for p in range(BL // 2):
                hp = p3p.tile([NBW, NTW], BF16, name="hp", tag="hp", bufs=3)
                nc.sync.dma_start(out=hp,
                                  in_=d["hT"][2 * p:2 * p + 2, :, sl])
                hps.append(hp)

        pas = []
        for p in range(BL // 2):
            b0, b1 = 2 * p, 2 * p + 1
            pa = psacc.tile([NBW, NTW], F32, name="pa2", tag="acc")
            # first matmul of the group must span all 128 partitions with
            # start=True (start zeroes the whole bank region); the two
            # half-partition identity matmuls then accumulate.
            amain = [lambda: nc.tensor.matmul(
                         pa, yc[p][:, 0:2, 0, :], a1[:, 0:2, :],
                         start=True, stop=False, perf_mode=DR),
                     lambda: nc.tensor.matmul(
                         pa[0:U, :], kc0, xT[b0][:, sl],
                         start=False, stop=False),
                     lambda: nc.tensor.matmul(
                         pa[U:2 * U, :], kc0, xT[b1][:, sl],
                         start=False, stop=False)]
            for m, asl in ((0, a1), (1, a2)):
                for jp in range(NJP):
                    if m == 0 and jp == 0:
                        continue
                    def f(m=m, asl=asl, jp=jp, p=p, pa=pa):
                        js = slice(2 * jp, 2 * jp + 2)
                        nc.tensor.matmul(pa, yc[p][:, js, m, :],
                                         asl[:, js, :], start=False,
                                         stop=(m == 1 and jp == NJP - 1),
                                         perf_mode=DR)
                    amain.append(f)
            extra = yc1_rest if (t == 0 and p == 0) else []
            _interleave(amain, extra, ratio=3)
            pas.append(pa)

        # prefetch next tile's h while this tile's B-groups run
        if t + 1 < NT:
            nhps = []
            nsl = slice((t + 1) * NTW, (t + 2) * NTW)
            for p in range(BL // 2):
                hp = p3p.tile([NBW, NTW], BF16, name="hp", tag="hp", bufs=3)
                nc.sync.dma_start(out=hp,
                                  in_=d["hT"][2 * p:2 * p + 2, :, nsl])
                nhps.append(hp)

        pbs, tmps, ssums, cts = [], [], [], []
        for p in range(BL // 2):
            pb = psb.tile([NBW, NTW], F32, name="pb2", tag="pb")
            for jp in range(NJP):
                js = slice(2 * jp, 2 * jp + 2)
                nc.tensor.matmul(pb, yc[p][:, js, 2, :], et[t][:, js, :],
                                 start=(jp == 0), stop=(jp == NJP - 1),
                                 perf_mode=DR)
            pbs.append(pb)
            tmp = stage.tile([NBW, NTW], BF16, name="tmp2", tag="tmp",
                             bufs=4)
            nc.vector.tensor_mul(tmp, pb, rdbc[t])
            tmps.append(tmp)
        for p in range(BL // 2):
            ssum = stage.tile([NBW, NTW], BF16, name="ssum2", tag="ssum",
                              bufs=4)
            nc.vector.tensor_add(ssum, pas[p], tmps[p])
            ssums.append(ssum)
        for p in range(BL // 2):
            # out holds 2*h_new = (h-c)*tu + (h+c); the host halves it.
            # plain tensor_tensor ops get the DVE 2x 16-bit mode, unlike
            # scalar_tensor_tensor which runs at 1x.
            ct = stage.tile([NBW, NTW], BF16, name="ct", tag="ct")
            nc.scalar.activation(ct, ssums[p], AF.Tanh, scale=GSCALE,
                                 bias=bc2)
            cts.append(ct)
        d2s, s2s = [], []
        for p in range(BL // 2):
            d2 = stage.tile([NBW, NTW], BF16, name="d2", tag="d2")
            nc.vector.tensor_sub(d2, hps[p], cts[p])
            d2s.append(d2)
            s2 = stage.tile([NBW, NTW], BF16, name="s2", tag="s2")
            nc.vector.tensor_add(s2, hps[p], cts[p])
            s2s.append(s2)
        for p in range(BL // 2):
            nc.vector.tensor_mul(d2s[p], d2s[p], ut[p][:, sl])
            t1 = p3p.tile([NBW, NTW], BF16, name="t1", tag="t1")
            nc.vector.tensor_add(t1, d2s[p], s2s[p])
            nc.sync.dma_start(out=out_h[2 * p:2 * p + 2, :, sl], in_=t1)
        if t + 1 < NT:
            hps = nhps

    ctx.close()


_CACHE = {}


def _get_program():
    if "nc" not in _CACHE:
        _CACHE["nc"] = _build_program()
    return _CACHE["nc"]


def _prep_inputs(inputs, h_prev, adj1, adj2, feat, SE, Wq, Wk, Ws1, bs1, Ws2,
                 bs2, r_kernel, r_bias, u_kernel, u_bias, c_kernel, c_bias):
    bf = ml_dtypes.bfloat16
    f8 = ml_dtypes.float8_e4m3
    f32 = np.float32
    perm = list(range(DIN, FROWS)) + list(range(DIN))  # [h(64); inputs(2)]

    h3 = np.asarray(h_prev, f32).reshape(B, N, U)
    hT = np.ascontiguousarray(h3.transpose(0, 2, 1))            # [B, U, N]
    inT = np.asarray(inputs, f32).transpose(0, 2, 1)            # [B, DIN, N]
    xT = np.concatenate([hT, inT], axis=1).astype(bf)           # [B, 66, N]
    hTb = hT.astype(bf)

    rk = np.asarray(r_kernel, f32)[:, perm, :]
    uk = np.asarray(u_kernel, f32)[:, perm, :]
    ck = np.asarray(c_kernel, f32)[:, perm, :]
    kkall = (BETA * np.concatenate(
        [np.concatenate([rk[m], uk[m]], axis=1) for m in (1, 2, 3)],
        axis=1)).astype(bf)                                     # [66, 384]
    kk0 = (ALPHA * BETA * np.concatenate([rk[0], uk[0]], axis=1)).astype(bf)
    # xT h-rows hold (1+tanh)*h = 2*r*h after phase 1; halve the c-kernel
    # h-rows to compensate
    ck = ck.copy()
    ck[:, 0:U, :] *= 0.5
    kcall = (BETA * np.concatenate([ck[1], ck[2], ck[3]], axis=1)).astype(bf)
    kc0 = (ALPHA * BETA * ck[0]).astype(bf)

    def adj_tiles(a):
        aT = np.asarray(a, f32).T * ALPHA
        np.clip(aT, -240.0, 240.0, out=aT)
        q = aT.astype(f8)                                      # [n, m] = A^T
        # tile[t, p, j, w] = A^T[j*128 + p, t*512 + w]
        return np.ascontiguousarray(
            q.reshape(NB, NBW, NT, NTW).transpose(2, 1, 0, 3))

    shared = {
        "a1q": adj_tiles(adj1),
        "a2q": adj_tiles(adj2),
        "fsT": np.ascontiguousarray(
            np.concatenate([np.asarray(feat, f32).T, np.asarray(SE, f32).T],
                           axis=0)).astype(bf),
        "wq": np.asarray(Wq, f32).astype(bf),
        "wk": np.asarray(Wk, f32).astype(bf),
        "ws1": np.asarray(Ws1, f32).astype(bf),
        "bs1v": np.asarray(bs1, f32).reshape(U, 1),
        "ws2a": (ALPHA * np.asarray(Ws2, f32)).reshape(U, 1).astype(bf),
        "bs2a": (ALPHA * np.asarray(bs2, f32)).reshape(1, 1),
        "kkall": kkall,
        "kk0": kk0,
        "kcall": kcall,
        "kc0": kc0,
        "bru2": 0.5 * np.concatenate(
            [np.asarray(r_bias, f32).mean(0),
             np.asarray(u_bias, f32).mean(0)]).reshape(-1, 1),
        "bc2": np.tile(np.asarray(c_bias, f32).mean(0), 2).reshape(-1, 1),
    }
    in_maps = []
    for c in range(NCORES):
        bsl = slice(c * BL, (c + 1) * BL)
        m = dict(shared)
        m["xT"] = np.ascontiguousarray(xT[bsl])
        m["hT"] = np.ascontiguousarray(hTb[bsl])
        in_maps.append(m)
    return in_maps


def kernel(**inputs):
    os.environ.setdefault("NEURON_RT_RESET_CORES", "1")
    nc = _get_program()
    in_maps = _prep_inputs(**inputs)
    res = None
    err = None
    for _ in range(2):
        try:
            res = run_bass_kernel_spmd(nc, in_maps, list(range(NCORES)))
            break
        except Exception as e:  # e.g. a wedged device; retry once
            err = e
    if res is None:
        raise err
    outs = []
    for c in range(NCORES):
        o = np.asarray(res.results[c]["out"], dtype=np.float32)
        outs.append(0.5 * o.transpose(0, 2, 1).reshape(BL, N * U))
    return np.concatenate(outs, axis=0).astype(np.float32)


# revision 32
# speedup vs baseline: 2.3486x; 1.0029x over previous
"""MFGCGRU (graph-conv GRU cell) Trainium2 kernel — fp8 DoubleRow edition.

Strategy: data-parallel over batch B=32 across 8 NeuronCores (4 per core).
The diffusion conv is computed kernel-first (S @ (X k)) with the node
contraction run as fp8e4 DoubleRow matmuls (2 node-blocks of 128 per PE
instruction at 0.5 cycles/row): stationary Y-pair [128, 2, 128] fp8,
moving support-pair [128, 2, 512] fp8 (1024-wide moving).

Precision plan (rel-err ~1.3e-2 < 2e-2 on HW):
  - x activations, GRU kernels, identity-support matmul, Y-generation all
    bf16; only the big node-contraction operands (supports + Y) are fp8.
  - supports scaled by ALPHA=64 (adj on host; e via exp ln-bias; the
    sentinel row via host-scaled Ws2/bs2), Y scaled by BETA=16 (host-scaled
    kernels); identity kernels carry ALPHA*BETA; the gate activations
    descale by 0.25/(ALPHA*BETA).

Scheduling: the whole program uses only the `exp_and_others` activation
table — both GRU sigmoids are computed as (1+tanh(z/2))/2 with the /2 and
+1 folded into host-side constants — so the ACT engine never reloads its
function table.  e^T generation for tile t+1 is interleaved into tile t's
A-group matmuls so the PE never waits on the ACT exp evacuations; et tiles
(fp8) and the a1 support slices are cached in SBUF across both phases;
u (as tanh) is kept in SBUF instead of a DRAM round-trip.
"""

import contextlib
import os

import numpy as np
import ml_dtypes

import concourse.bass as bass
import concourse.bacc as bacc
import concourse.tile as tile
from concourse import mybir
from concourse.bass_utils import run_bass_kernel_spmd

F32 = mybir.dt.float32
BF16 = mybir.dt.bfloat16
FP8 = mybir.dt.float8e4
AF = mybir.ActivationFunctionType
DR = mybir.MatmulPerfMode.DoubleRow
OP = mybir.AluOpType

B, N, DIN, U, FD, SD = 32, 2048, 2, 64, 32, 64
NCORES = 8
BL = B // NCORES          # batches per core
NTW = 512                 # n-tile width (output columns per tile)
NT = N // NTW             # 4 n-tiles
NBW = 128                 # node-block width
NB = N // NBW             # 16 node blocks
NJP = NB // 2             # 8 node-block pairs (DoubleRow)
FROWS = DIN + U           # 66

ALPHA = 64.0              # support scale
BETA = 16.0               # Y scale
GSCALE = 0.25 / (ALPHA * BETA)   # gate pre-activation descale (incl /M)
GS2 = GSCALE / 2.0               # tanh-form sigmoid input scale


def _build_program():
    nc = bacc.Bacc("TRN2", debug=False, num_devices=NCORES)

    d = {}

    def din(name, shape, dt):
        d[name] = nc.dram_tensor(name, shape, dt, kind="ExternalInput").ap()

    din("xT", [BL, FROWS, N], BF16)
    din("hT", [BL, U, N], BF16)
    din("a1q", [NT, NBW, NB, NTW], FP8)
    din("a2q", [NT, NBW, NB, NTW], FP8)
    din("fsT", [FD + SD, N], BF16)
    din("wq", [FD, U], BF16)
    din("wk", [FD, U], BF16)
    din("ws1", [FD + SD, U], BF16)
    din("bs1v", [U, 1], F32)
    din("ws2a", [U, 1], BF16)
    din("bs2a", [1, 1], F32)
    din("kkall", [FROWS, 3 * 2 * U], BF16)
    din("kk0", [FROWS, 2 * U], BF16)
    din("kcall", [FROWS, 3 * U], BF16)
    din("kc0", [FROWS, U], BF16)
    din("bru2", [2 * U, 1], F32)
    din("bc2", [2 * U, 1], F32)
    out_h = nc.dram_tensor("out", [BL, U, N], BF16, kind="ExternalOutput").ap()

    with tile.TileContext(nc) as tc:
        _emit(tc, d, out_h)
    nc.compile()
    return nc


def _interleave(main, extra, ratio=4):
    """Emit `ratio` thunks from main per one from extra."""
    mi = ei = 0
    while mi < len(main) or ei < len(extra):
        for _ in range(ratio):
            if mi < len(main):
                main[mi]()
                mi += 1
        if ei < len(extra):
            extra[ei]()
            ei += 1


def _emit(tc, d, out_h):
    nc = tc.nc
    ctx = contextlib.ExitStack()
    const = ctx.enter_context(tc.tile_pool(name="const", bufs=1))
    persist = ctx.enter_context(tc.tile_pool(name="persist", bufs=1))
    stage = ctx.enter_context(tc.tile_pool(name="stage", bufs=2))
    p3p = ctx.enter_context(tc.tile_pool(name="p3p", bufs=2))
    psacc = ctx.enter_context(tc.tile_pool(name="psacc", bufs=4, space="PSUM"))
    psscr = ctx.enter_context(tc.tile_pool(name="psscr", bufs=2, space="PSUM"))
    psb = ctx.enter_context(tc.tile_pool(name="psb", bufs=2, space="PSUM"))

    # ---- constants / weights in SBUF ----
    def cload(name):
        ap = d[name]
        t = const.tile(list(ap.shape), ap.dtype, name=f"c_{name}")
        nc.sync.dma_start(out=t, in_=ap)
        return t

    fsT = cload("fsT")
    wq = cload("wq")
    wk = cload("wk")
    ws1 = cload("ws1")
    bs1v = cload("bs1v")
    ws2a = cload("ws2a")
    bs2a = cload("bs2a")
    kkall = cload("kkall")
    kk0 = cload("kk0")
    kcall = cload("kcall")
    kc0 = cload("kc0")
    bru2 = cload("bru2")
    bc2 = cload("bc2")

    ones2 = const.tile([NBW, 2, 16], FP8, name="ones2")
    nc.vector.memset(ones2, 1.0)
    ones_row = const.tile([1, NBW], BF16, name="ones_row")
    nc.vector.memset(ones_row, ALPHA)
    lnal = const.tile([NBW, 1], F32, name="lnal")
    nc.vector.memset(lnal, float(np.log(ALPHA)))

    # ---- persistent activations ----
    xT = [persist.tile([FROWS, N], BF16, name=f"xT{b}", tag=f"xT{b}")
          for b in range(BL)]
    nc.sync.dma_start(out=xT[0], in_=d["xT"][0])

    QT = persist.tile([FD, 2, N], FP8, name="QT", tag="QT")
    KT = persist.tile([FD, 2, N], FP8, name="KT", tag="KT")
    s_row = persist.tile([1, N], BF16, name="s_row", tag="s_row")
    rdbc = [persist.tile([NBW, NTW], BF16, name=f"rdbc{t}", tag=f"rdbc{t}")
            for t in range(NT)]
    et = [persist.tile([NBW, NB, NTW], FP8, name=f"et{t}", tag=f"et{t}")
          for t in range(NT)]
    a1c = [persist.tile([NBW, NB, NTW], FP8, name=f"a1c{t}", tag=f"a1c{t}")
           for t in range(NT)]
    a2c = [persist.tile([NBW, NB, NTW], FP8, name=f"a2c{t}", tag=f"a2c{t}")
           for t in range(NT)]
    ut = [persist.tile([NBW, N], BF16, name=f"ut{p}", tag=f"ut{p}")
          for p in range(BL // 2)]
    y = [persist.tile([NBW, NB, 3, 2 * U], FP8, name=f"y_{b}", tag=f"y{b}")
         for b in range(BL)]
    yc = [persist.tile([NBW, NB, 3, 2 * U], FP8, name=f"yc_{p}", tag=f"yc{p}")
          for p in range(BL // 2)]

    # adjacency DMAs for phase-1 t=0 (a1 cached for both phases; a2
    # streamed); xT[0] already queued so ygen(b0) can start, adjacency
    # next so A(t0) isn't DMA-starved, remaining xT after
    nc.sync.dma_start(out=a1c[0], in_=d["a1q"][0])
    nc.sync.dma_start(out=a2c[0], in_=d["a2q"][0])
    for b in range(1, BL):
        nc.sync.dma_start(out=xT[b], in_=d["xT"][b])

    # ---- thunk generators ----
    def prelude_thunks():
        th = []
        for t in range(NT):
            sl = slice(t * NTW, (t + 1) * NTW)
            # KT before QT: the interleaved e-gen(t0) thunk ei fires after
            # main[4*ei+3]; ei=0 needs all four K/Q writes of the t0 block
            # already emitted (Tile cannot depend on future instructions)
            for dst, w in ((KT, wk), (QT, wq)):
                for uh in range(2):
                    def f(dst=dst, w=w, uh=uh, sl=sl):
                        pq = psacc.tile([FD, NTW], F32, name="pq", tag="acc")
                        nc.tensor.matmul(pq, w[:, uh * FD:(uh + 1) * FD],
                                         fsT[0:FD, sl], start=True, stop=True)
                        if uh == 0:
                            nc.scalar.activation(dst[:, uh, sl], pq, AF.Relu)
                        else:
                            nc.vector.tensor_scalar_max(dst[:, uh, sl], pq, 0.0)
                    th.append(f)

            def g(sl=sl):
                ps1 = psacc.tile([U, NTW], F32, name="ps1", tag="acc")
                nc.tensor.matmul(ps1, ws1, fsT[:, sl], start=True, stop=True)
                s1t = stage.tile([U, NTW], BF16, name="s1t", tag="s1t", bufs=1)
                nc.scalar.activation(s1t, ps1, AF.Relu, bias=bs1v)
                ps2 = psacc.tile([1, NTW], F32, name="ps2", tag="acc")
                nc.tensor.matmul(ps2, ws2a, s1t, start=True, stop=True)
                nc.scalar.activation(s_row[:, sl], ps2, AF.Relu, bias=bs2a)
            th.append(g)
        return th

    def ygen_thunks(b):
        th = []
        for j in range(NB):
            def f(b=b, j=j):
                nsl = slice(j * NBW, (j + 1) * NBW)
                py = psscr.tile([NBW, 3 * 2 * U], F32, name="py", tag="scr")
                nc.tensor.matmul(py, xT[b][:, nsl], kkall,
                                 start=True, stop=True)
                src = py.rearrange("p (m u) -> p m u", m=3)
                if (b * NB + j) % 16 < 11:
                    nc.vector.tensor_copy(y[b][:, j, :, :], src)
                else:
                    nc.scalar.activation(y[b][:, j, :, :], src, AF.Copy)
            th.append(f)
        return th

    def ycgen_thunks(p):
        th = []
        for half in range(2):
            b = 2 * p + half
            usl = slice(half * U, (half + 1) * U)
            for j0 in range(0, NB, 2):
                def f(b=b, usl=usl, j0=j0, p=p):
                    pool_ = psscr if (j0 // 2) % 2 == 0 else psb
                    tag_ = "scr" if (j0 // 2) % 2 == 0 else "pb"
                    pyc = pool_.tile([NBW, 2 * 3 * U], F32, name="pyc",
                                     tag=tag_)
                    for i in range(2):
                        nsl = slice((j0 + i) * NBW, (j0 + i + 1) * NBW)
                        # start zeroes the whole PSUM bank region: only the
                        # first matmul of the pair may set it
                        nc.tensor.matmul(pyc[:, i * 3 * U:(i + 1) * 3 * U],
                                         xT[b][:, nsl], kcall,
                                         start=(i == 0), stop=(i == 1))
                    src = pyc.rearrange("p (j m u) -> p j m u", j=2, m=3)
                    if (b * NB + j0) % 4 < 1:
                        nc.vector.tensor_copy(yc[p][:, j0:j0 + 2, :, usl], src)
                    else:
                        nc.scalar.activation(yc[p][:, j0:j0 + 2, :, usl], src,
                                             AF.Copy)
                th.append(f)
        return th

    def egen_thunks(t):
        sl = slice(t * NTW, (t + 1) * NTW)
        th = []
        for j in range(NB):
            def f(j=j, t=t, sl=sl):
                pe = psscr.tile([NBW, NTW], F32, name="pe", tag="scr")
                nc.tensor.matmul(pe, KT[:, :, j * NBW:(j + 1) * NBW],
                                 QT[:, :, sl], start=True, stop=True,
                                 perf_mode=DR)
                nc.scalar.activation(et[t][:, j, :], pe, AF.Exp, scale=0.125,
                                     bias=lnal)
            th.append(f)
        return th

    def agroup1_thunks(b, t, a1, a2, pa):
        sl = slice(t * NTW, (t + 1) * NTW)
        head = [lambda: nc.tensor.matmul(pa, kk0, xT[b][:, sl],
                                         start=True, stop=False)]
        tail = []
        for m, asl, out in ((0, a1, head), (1, a2, tail)):
            for jp in range(NJP):
                def f(m=m, asl=asl, jp=jp, b=b):
                    js = slice(2 * jp, 2 * jp + 2)
                    nc.tensor.matmul(pa, y[b][:, js, m, :], asl[:, js, :],
                                     start=False,
                                     stop=(m == 1 and jp == NJP - 1),
                                     perf_mode=DR)
                out.append(f)
        return head, tail

    def dblock(t):
        sl = slice(t * NTW, (t + 1) * NTW)
        pd = psb.tile([1, NTW], F32, name="pd", tag="pb")
        for jp in range(NJP):
            js = slice(2 * jp, 2 * jp + 2)
            nc.tensor.matmul(pd, ones2[:, :, 0:1], et[t][:, js, :],
                             start=(jp == 0), stop=(jp == NJP - 1),
                             perf_mode=DR)
        dsb = stage.tile([1, NTW], F32, name="dsb", tag="dsb", bufs=1)
        nc.vector.tensor_add(dsb, pd, s_row[:, sl])
        rds = stage.tile([1, NTW], BF16, name="rds", tag="rds", bufs=1)
        with nc.allow_low_precision(reason="rdbc is bf16 by design"):
            nc.vector.reciprocal(rds, dsb)
        pr = psb.tile([NBW, NTW], F32, name="pr", tag="pb")
        nc.tensor.matmul(pr, ones_row, rds, start=True, stop=True)
        nc.vector.tensor_copy(rdbc[t], pr)

    def bgroup1_all(t, pas):
        """B-matmuls then a breadth-first epilogue: each DVE stage for all
        four batches before the next, so the strict-FIFO DVE queue never
        head-blocks on an ACT result while later batches' work is ready.
        pb ring is 2 deep: tmp(b) is emitted right when pb(b+2) needs the
        slot back."""
        sl = slice(t * NTW, (t + 1) * NTW)
        pbs, tmps, ssums, ths = [], [], [], []
        for b in range(BL):
            pb = psb.tile([NBW, NTW], F32, name="pb", tag="pb")
            for jp in range(NJP):
                js = slice(2 * jp, 2 * jp + 2)
                nc.tensor.matmul(pb, y[b][:, js, 2, :], et[t][:, js, :],
                                 start=(jp == 0), stop=(jp == NJP - 1),
                                 perf_mode=DR)
            pbs.append(pb)
            tmp = stage.tile([NBW, NTW], BF16, name="tmp", tag="tmp",
                             bufs=4)
            nc.vector.tensor_mul(tmp, pb, rdbc[t])
            tmps.append(tmp)
        for b in range(BL):
            ssum = stage.tile([NBW, NTW], BF16, name="ssum", tag="ssum",
                              bufs=4)
            nc.vector.tensor_add(ssum, pas[b], tmps[b])
            ssums.append(ssum)
        for b in range(BL):
            # th = tanh(GS2*ssum + bru/2): rows 0:64 -> (1+th)*h into xT
            # h-rows (factor 2 absorbed in host-halved kcall/kc0 h-rows);
            # rows 64:128 -> ut stores tanh-form u
            th = stage.tile([NBW, NTW], BF16, name="th", tag="th", bufs=4)
            nc.scalar.activation(th, ssums[b], AF.Tanh, scale=GS2,
                                 bias=bru2)
            ths.append(th)
        for b in range(BL):
            nc.vector.scalar_tensor_tensor(
                xT[b][0:U, sl], ths[b][0:U, :], 1.0, xT[b][0:U, sl],
                op0=OP.add, op1=OP.mult)
        for b in range(BL):
            p, half = b // 2, b % 2
            nc.vector.tensor_copy(ut[p][half * U:(half + 1) * U, sl],
                                  ths[b][U:2 * U, :])

    # ==================== pre-phase ====================
    pre = prelude_thunks() + ygen_thunks(0) + ygen_thunks(1)
    _interleave(pre, egen_thunks(0), ratio=6)

    # ==================== phase 1: r & u gates ====================
    for t in range(NT):
        sl = slice(t * NTW, (t + 1) * NTW)
        a1, a2 = a1c[t], a2c[t]

        pas = []
        heads, tails = [], []
        for b in range(BL):
            pa = psacc.tile([NBW, NTW], F32, name="pa", tag="acc")
            hd, tl = agroup1_thunks(b, t, a1, a2, pa)
            heads.extend(hd)
            tails.extend(tl)
            pas.append(pa)
            if t == 0 and b == 0:
                heads.extend(ygen_thunks(2))
            if t == 0 and b == 1:
                heads.extend(ygen_thunks(3))
        # id + a1 contributions for every batch first: at t=0 the a2 slice
        # is still streaming in while these run
        _interleave(heads + tails,
                    egen_thunks(t + 1) if t + 1 < NT else [], ratio=4)

        if t + 1 < NT:
            nc.sync.dma_start(out=a1c[t + 1], in_=d["a1q"][t + 1])
            nc.sync.dma_start(out=a2c[t + 1], in_=d["a2q"][t + 1])

        if t == 0:
            dblock(0)
        bgroup1_all(t, pas)
        # next tile's normalizer now: its exps completed during this tile's
        # A-groups, and emitting it here keeps rdbc(t+1) clear of the
        # B-epilogue critical path
        if t + 1 < NT:
            dblock(t + 1)

    # ==================== phase 2: c gate & h_new ====================
    # node-blocks j<12 depend only on t0..t2 epilogues — emit them first so
    # the PE keeps busy while the t3 epilogue chain drains; yc[1]'s late
    # blocks are interleaved into phase-2 t0's first A-group
    yc0, yc1 = ycgen_thunks(0), ycgen_thunks(1)
    for f in [f for i, f in enumerate(yc0) if (i % 8) < 6]:
        f()
    for f in [f for i, f in enumerate(yc0) if (i % 8) >= 6]:
        f()
    for f in [f for i, f in enumerate(yc1) if (i % 8) < 6]:
        f()
    yc1_rest = [f for i, f in enumerate(yc1) if (i % 8) >= 6]

    for t in range(NT):
        sl = slice(t * NTW, (t + 1) * NTW)
        a1, a2 = a1c[t], a2c[t]

        if t == 0:
            hps = []
            for p in range(BL // 2):
                hp = p3p.tile([NBW, NTW], BF16, name="hp", tag="hp", bufs=3)
                nc.sync.dma_start(out=hp,
                                  in_=d["hT"][2 * p:2 * p + 2, :, sl])
                hps.append(hp)

        pas = []
        for p in range(BL // 2):
            b0, b1 = 2 * p, 2 * p + 1
            pa = psacc.tile([NBW, NTW], F32, name="pa2", tag="acc")
            # first matmul of the group must span all 128 partitions with
            # start=True (start zeroes the whole bank region); the two
            # half-partition identity matmuls then accumulate.
            amain = [lambda: nc.tensor.matmul(
                         pa, yc[p][:, 0:2, 0, :], a1[:, 0:2, :],
                         start=True, stop=False, perf_mode=DR),
                     lambda: nc.tensor.matmul(
                         pa[0:U, :], kc0, xT[b0][:, sl],
                         start=False, stop=False),
                     lambda: nc.tensor.matmul(
                         pa[U:2 * U, :], kc0, xT[b1][:, sl],
                         start=False, stop=False)]
            for m, asl in ((0, a1), (1, a2)):
                for jp in range(NJP):
                    if m == 0 and jp == 0:
                        continue
                    def f(m=m, asl=asl, jp=jp, p=p, pa=pa):
                        js = slice(2 * jp, 2 * jp + 2)
                        nc.tensor.matmul(pa, yc[p][:, js, m, :],
                                         asl[:, js, :], start=False,
                                         stop=(m == 1 and jp == NJP - 1),
                                         perf_mode=DR)
                    amain.append(f)
            extra = yc1_rest if (t == 0 and p == 0) else []
            _interleave(amain, extra, ratio=3)
            pas.append(pa)

        # prefetch next tile's h while this tile's B-groups run
        if t + 1 < NT:
            nhps = []
            nsl = slice((t + 1) * NTW, (t + 2) * NTW)
            for p in range(BL // 2):
                hp = p3p.tile([NBW, NTW], BF16, name="hp", tag="hp", bufs=3)
                nc.sync.dma_start(out=hp,
                                  in_=d["hT"][2 * p:2 * p + 2, :, nsl])
                nhps.append(hp)

        pbs, tmps, ssums, cts = [], [], [], []
        for p in range(BL // 2):
            pb = psb.tile([NBW, NTW], F32, name="pb2", tag="pb")
            for jp in range(NJP):
                js = slice(2 * jp, 2 * jp + 2)
                nc.tensor.matmul(pb, yc[p][:, js, 2, :], et[t][:, js, :],
                                 start=(jp == 0), stop=(jp == NJP - 1),
                                 perf_mode=DR)
            pbs.append(pb)
            tmp = stage.tile([NBW, NTW], BF16, name="tmp2", tag="tmp",
                             bufs=4)
            nc.vector.tensor_mul(tmp, pb, rdbc[t])
            tmps.append(tmp)
        for p in range(BL // 2):
            ssum = stage.tile([NBW, NTW], BF16, name="ssum2", tag="ssum",
                              bufs=4)
            nc.vector.tensor_add(ssum, pas[p], tmps[p])
            ssums.append(ssum)
        for p in range(BL // 2):
            # out holds 2*h_new = (h-c)*tu + (h+c); the host halves it.
            # plain tensor_tensor ops get the DVE 2x 16-bit mode, unlike
            # scalar_tensor_tensor which runs at 1x.
            ct = stage.tile([NBW, NTW], BF16, name="ct", tag="ct")
            nc.scalar.activation(ct, ssums[p], AF.Tanh, scale=GSCALE,
                                 bias=bc2)
            cts.append(ct)
        d2s, s2s = [], []
        for p in range(BL // 2):
            d2 = stage.tile([NBW, NTW], BF16, name="d2", tag="d2")
            nc.vector.tensor_sub(d2, hps[p], cts[p])
            d2s.append(d2)
            s2 = stage.tile([NBW, NTW], BF16, name="s2", tag="s2")
            nc.gpsimd.tensor_add(s2, hps[p], cts[p])
            s2s.append(s2)
        for p in range(BL // 2):
            nc.vector.tensor_mul(d2s[p], d2s[p], ut[p][:, sl])
            t1 = p3p.tile([NBW, NTW], BF16, name="t1", tag="t1")
            nc.vector.tensor_add(t1, d2s[p], s2s[p])
            nc.sync.dma_start(out=out_h[2 * p:2 * p + 2, :, sl], in_=t1)
        if t + 1 < NT:
            hps = nhps

    ctx.close()


_CACHE = {}


def _get_program():
    if "nc" not in _CACHE:
        _CACHE["nc"] = _build_program()
    return _CACHE["nc"]


def _prep_inputs(inputs, h_prev, adj1, adj2, feat, SE, Wq, Wk, Ws1, bs1, Ws2,
                 bs2, r_kernel, r_bias, u_kernel, u_bias, c_kernel, c_bias):
    bf = ml_dtypes.bfloat16
    f8 = ml_dtypes.float8_e4m3
    f32 = np.float32
    perm = list(range(DIN, FROWS)) + list(range(DIN))  # [h(64); inputs(2)]

    h3 = np.asarray(h_prev, f32).reshape(B, N, U)
    hT = np.ascontiguousarray(h3.transpose(0, 2, 1))            # [B, U, N]
    inT = np.asarray(inputs, f32).transpose(0, 2, 1)            # [B, DIN, N]
    xT = np.concatenate([hT, inT], axis=1).astype(bf)           # [B, 66, N]
    hTb = hT.astype(bf)

    rk = np.asarray(r_kernel, f32)[:, perm, :]
    uk = np.asarray(u_kernel, f32)[:, perm, :]
    ck = np.asarray(c_kernel, f32)[:, perm, :]
    kkall = (BETA * np.concatenate(
        [np.concatenate([rk[m], uk[m]], axis=1) for m in (1, 2, 3)],
        axis=1)).astype(bf)                                     # [66, 384]
    kk0 = (ALPHA * BETA * np.concatenate([rk[0], uk[0]], axis=1)).astype(bf)
    # xT h-rows hold (1+tanh)*h = 2*r*h after phase 1; halve the c-kernel
    # h-rows to compensate
    ck = ck.copy()
    ck[:, 0:U, :] *= 0.5
    kcall = (BETA * np.concatenate([ck[1], ck[2], ck[3]], axis=1)).astype(bf)
    kc0 = (ALPHA * BETA * ck[0]).astype(bf)

    def adj_tiles(a):
        aT = np.asarray(a, f32).T * ALPHA
        np.clip(aT, -240.0, 240.0, out=aT)
        q = aT.astype(f8)                                      # [n, m] = A^T
        # tile[t, p, j, w] = A^T[j*128 + p, t*512 + w]
        return np.ascontiguousarray(
            q.reshape(NB, NBW, NT, NTW).transpose(2, 1, 0, 3))

    shared = {
        "a1q": adj_tiles(adj1),
        "a2q": adj_tiles(adj2),
        "fsT": np.ascontiguousarray(
            np.concatenate([np.asarray(feat, f32).T, np.asarray(SE, f32).T],
                           axis=0)).astype(bf),
        "wq": np.asarray(Wq, f32).astype(bf),
        "wk": np.asarray(Wk, f32).astype(bf),
        "ws1": np.asarray(Ws1, f32).astype(bf),
        "bs1v": np.asarray(bs1, f32).reshape(U, 1),
        "ws2a": (ALPHA * np.asarray(Ws2, f32)).reshape(U, 1).astype(bf),
        "bs2a": (ALPHA * np.asarray(bs2, f32)).reshape(1, 1),
        "kkall": kkall,
        "kk0": kk0,
        "kcall": kcall,
        "kc0": kc0,
        "bru2": 0.5 * np.concatenate(
            [np.asarray(r_bias, f32).mean(0),
             np.asarray(u_bias, f32).mean(0)]).reshape(-1, 1),
        "bc2": np.tile(np.asarray(c_bias, f32).mean(0), 2).reshape(-1, 1),
    }
    in_maps = []
    for c in range(NCORES):
        bsl = slice(c * BL, (c + 1) * BL)
        m = dict(shared)
        m["xT"] = np.ascontiguousarray(xT[bsl])
        m["hT"] = np.ascontiguousarray(hTb[bsl])
        in_maps.append(m)
    return in_maps


def kernel(**inputs):
    os.environ.setdefault("NEURON_RT_RESET_CORES", "1")
    nc = _get_program()
    in_maps = _prep_inputs(**inputs)
    res = None
    err = None
    for _ in range(2):
        try:
            res = run_bass_kernel_spmd(nc, in_maps, list(range(NCORES)))
            break
        except Exception as e:  # e.g. a wedged device; retry once
            err = e
    if res is None:
        raise err
    outs = []
    for c in range(NCORES):
        o = np.asarray(res.results[c]["out"], dtype=np.float32)
        outs.append(0.5 * o.transpose(0, 2, 1).reshape(BL, N * U))
    return np.concatenate(outs, axis=0).astype(np.float32)


# revision 34
# speedup vs baseline: 2.3496x; 1.0005x over previous
"""MFGCGRU (graph-conv GRU cell) Trainium2 kernel — fp8 DoubleRow edition.

Strategy: data-parallel over batch B=32 across 8 NeuronCores (4 per core).
The diffusion conv is computed kernel-first (S @ (X k)) with the node
contraction run as fp8e4 DoubleRow matmuls (2 node-blocks of 128 per PE
instruction at 0.5 cycles/row): stationary Y-pair [128, 2, 128] fp8,
moving support-pair [128, 2, 512] fp8 (1024-wide moving).

Precision plan (rel-err ~1.3e-2 < 2e-2 on HW):
  - x activations, GRU kernels, identity-support matmul, Y-generation all
    bf16; only the big node-contraction operands (supports + Y) are fp8.
  - supports scaled by ALPHA=64 (adj on host; e via exp ln-bias; the
    sentinel row via host-scaled Ws2/bs2), Y scaled by BETA=16 (host-scaled
    kernels); identity kernels carry ALPHA*BETA; the gate activations
    descale by 0.25/(ALPHA*BETA).

Scheduling: the whole program uses only the `exp_and_others` activation
table — both GRU sigmoids are computed as (1+tanh(z/2))/2 with the /2 and
+1 folded into host-side constants — so the ACT engine never reloads its
function table.  e^T generation for tile t+1 is interleaved into tile t's
A-group matmuls so the PE never waits on the ACT exp evacuations; et tiles
(fp8) and the a1 support slices are cached in SBUF across both phases;
u (as tanh) is kept in SBUF instead of a DRAM round-trip.
"""

import contextlib
import os

import numpy as np
import ml_dtypes

import concourse.bass as bass
import concourse.bacc as bacc
import concourse.tile as tile
from concourse import mybir
from concourse.bass_utils import run_bass_kernel_spmd

F32 = mybir.dt.float32
BF16 = mybir.dt.bfloat16
FP8 = mybir.dt.float8e4
AF = mybir.ActivationFunctionType
DR = mybir.MatmulPerfMode.DoubleRow
OP = mybir.AluOpType

B, N, DIN, U, FD, SD = 32, 2048, 2, 64, 32, 64
NCORES = 8
BL = B // NCORES          # batches per core
NTW = 512                 # n-tile width (output columns per tile)
NT = N // NTW             # 4 n-tiles
NBW = 128                 # node-block width
NB = N // NBW             # 16 node blocks
NJP = NB // 2             # 8 node-block pairs (DoubleRow)
FROWS = DIN + U           # 66

ALPHA = 64.0              # support scale
BETA = 16.0               # Y scale
GSCALE = 0.25 / (ALPHA * BETA)   # gate pre-activation descale (incl /M)
GS2 = GSCALE / 2.0               # tanh-form sigmoid input scale


def _build_program():
    nc = bacc.Bacc("TRN2", debug=False, num_devices=NCORES)

    d = {}

    def din(name, shape, dt):
        d[name] = nc.dram_tensor(name, shape, dt, kind="ExternalInput").ap()

    din("xT", [BL, FROWS, N], BF16)
    din("hT", [BL, U, N], BF16)
    din("a1q", [NT, NBW, NB, NTW], FP8)
    din("a2q", [NT, NBW, NB, NTW], FP8)
    din("fsT", [FD + SD, N], BF16)
    din("wq", [FD, U], BF16)
    din("wk", [FD, U], BF16)
    din("ws1", [FD + SD, U], BF16)
    din("bs1v", [U, 1], F32)
    din("ws2a", [U, 1], BF16)
    din("bs2a", [1, 1], F32)
    din("kkall", [FROWS, 3 * 2 * U], BF16)
    din("kk0", [FROWS, 2 * U], BF16)
    din("kcall", [FROWS, 3 * U], BF16)
    din("kc0", [FROWS, U], BF16)
    din("bru2", [2 * U, 1], F32)
    din("bc2", [2 * U, 1], F32)
    out_h = nc.dram_tensor("out", [BL, U, N], BF16, kind="ExternalOutput").ap()

    with tile.TileContext(nc) as tc:
        _emit(tc, d, out_h)
    nc.compile()
    return nc


def _interleave(main, extra, ratio=4):
    """Emit `ratio` thunks from main per one from extra."""
    mi = ei = 0
    while mi < len(main) or ei < len(extra):
        for _ in range(ratio):
            if mi < len(main):
                main[mi]()
                mi += 1
        if ei < len(extra):
            extra[ei]()
            ei += 1


def _emit(tc, d, out_h):
    nc = tc.nc
    ctx = contextlib.ExitStack()
    const = ctx.enter_context(tc.tile_pool(name="const", bufs=1))
    persist = ctx.enter_context(tc.tile_pool(name="persist", bufs=1))
    stage = ctx.enter_context(tc.tile_pool(name="stage", bufs=2))
    p3p = ctx.enter_context(tc.tile_pool(name="p3p", bufs=2))
    psacc = ctx.enter_context(tc.tile_pool(name="psacc", bufs=4, space="PSUM"))
    psscr = ctx.enter_context(tc.tile_pool(name="psscr", bufs=2, space="PSUM"))
    psb = ctx.enter_context(tc.tile_pool(name="psb", bufs=2, space="PSUM"))

    # ---- constants / weights in SBUF ----
    def cload(name):
        ap = d[name]
        t = const.tile(list(ap.shape), ap.dtype, name=f"c_{name}")
        nc.sync.dma_start(out=t, in_=ap)
        return t

    fsT = cload("fsT")
    wq = cload("wq")
    wk = cload("wk")
    ws1 = cload("ws1")
    bs1v = cload("bs1v")
    ws2a = cload("ws2a")
    bs2a = cload("bs2a")
    kkall = cload("kkall")
    kk0 = cload("kk0")
    kcall = cload("kcall")
    kc0 = cload("kc0")
    bru2 = cload("bru2")
    bc2 = cload("bc2")

    ones2 = const.tile([NBW, 2, 16], FP8, name="ones2")
    nc.vector.memset(ones2, 1.0)
    ones_row = const.tile([1, NBW], BF16, name="ones_row")
    nc.vector.memset(ones_row, ALPHA)
    lnal = const.tile([NBW, 1], F32, name="lnal")
    nc.vector.memset(lnal, float(np.log(ALPHA)))

    # ---- persistent activations ----
    xT = [persist.tile([FROWS, N], BF16, name=f"xT{b}", tag=f"xT{b}")
          for b in range(BL)]
    nc.sync.dma_start(out=xT[0], in_=d["xT"][0])

    QT = persist.tile([FD, 2, N], FP8, name="QT", tag="QT")
    KT = persist.tile([FD, 2, N], FP8, name="KT", tag="KT")
    s_row = persist.tile([1, N], BF16, name="s_row", tag="s_row")
    rdbc = [persist.tile([NBW, NTW], BF16, name=f"rdbc{t}", tag=f"rdbc{t}")
            for t in range(NT)]
    et = [persist.tile([NBW, NB, NTW], FP8, name=f"et{t}", tag=f"et{t}")
          for t in range(NT)]
    a1c = [persist.tile([NBW, NB, NTW], FP8, name=f"a1c{t}", tag=f"a1c{t}")
           for t in range(NT)]
    a2c = [persist.tile([NBW, NB, NTW], FP8, name=f"a2c{t}", tag=f"a2c{t}")
           for t in range(NT)]
    ut = [persist.tile([NBW, N], BF16, name=f"ut{p}", tag=f"ut{p}")
          for p in range(BL // 2)]
    y = [persist.tile([NBW, NB, 3, 2 * U], FP8, name=f"y_{b}", tag=f"y{b}")
         for b in range(BL)]
    yc = [persist.tile([NBW, NB, 3, 2 * U], FP8, name=f"yc_{p}", tag=f"yc{p}")
          for p in range(BL // 2)]

    # adjacency DMAs for phase-1 t=0 (a1 cached for both phases; a2
    # streamed); xT[0] already queued so ygen(b0) can start, adjacency
    # next so A(t0) isn't DMA-starved, remaining xT after
    # t0 adjacency arrives in j-block chunks so the first A-group matmuls
    # (which consume j-pairs in order) start before the full tile lands
    for h in range(4):
        jsl = slice(h * (NB // 4), (h + 1) * (NB // 4))
        nc.sync.dma_start(out=a1c[0][:, jsl, :], in_=d["a1q"][0][:, jsl, :])
    for h in range(2):
        jsl = slice(h * (NB // 2), (h + 1) * (NB // 2))
        nc.sync.dma_start(out=a2c[0][:, jsl, :], in_=d["a2q"][0][:, jsl, :])
    for b in range(1, BL):
        nc.sync.dma_start(out=xT[b], in_=d["xT"][b])

    # ---- thunk generators ----
    def prelude_thunks():
        th = []
        for t in range(NT):
            sl = slice(t * NTW, (t + 1) * NTW)
            # KT before QT: the interleaved e-gen(t0) thunk ei fires after
            # main[4*ei+3]; ei=0 needs all four K/Q writes of the t0 block
            # already emitted (Tile cannot depend on future instructions)
            for dst, w in ((KT, wk), (QT, wq)):
                for uh in range(2):
                    def f(dst=dst, w=w, uh=uh, sl=sl):
                        pq = psacc.tile([FD, NTW], F32, name="pq", tag="acc")
                        nc.tensor.matmul(pq, w[:, uh * FD:(uh + 1) * FD],
                                         fsT[0:FD, sl], start=True, stop=True)
                        if uh == 0:
                            nc.scalar.activation(dst[:, uh, sl], pq, AF.Relu)
                        else:
                            nc.vector.tensor_scalar_max(dst[:, uh, sl], pq, 0.0)
                    th.append(f)

            def g(sl=sl):
                ps1 = psacc.tile([U, NTW], F32, name="ps1", tag="acc")
                nc.tensor.matmul(ps1, ws1, fsT[:, sl], start=True, stop=True)
                s1t = stage.tile([U, NTW], BF16, name="s1t", tag="s1t", bufs=1)
                nc.scalar.activation(s1t, ps1, AF.Relu, bias=bs1v)
                ps2 = psacc.tile([1, NTW], F32, name="ps2", tag="acc")
                nc.tensor.matmul(ps2, ws2a, s1t, start=True, stop=True)
                nc.scalar.activation(s_row[:, sl], ps2, AF.Relu, bias=bs2a)
            th.append(g)
        return th

    def ygen_thunks(b):
        th = []
        for j in range(NB):
            def f(b=b, j=j):
                nsl = slice(j * NBW, (j + 1) * NBW)
                py = psscr.tile([NBW, 3 * 2 * U], F32, name="py", tag="scr")
                nc.tensor.matmul(py, xT[b][:, nsl], kkall,
                                 start=True, stop=True)
                src = py.rearrange("p (m u) -> p m u", m=3)
                if (b * NB + j) % 16 < 11:
                    nc.vector.tensor_copy(y[b][:, j, :, :], src)
                else:
                    nc.scalar.activation(y[b][:, j, :, :], src, AF.Copy)
            th.append(f)
        return th

    def ycgen_thunks(p):
        th = []
        for half in range(2):
            b = 2 * p + half
            usl = slice(half * U, (half + 1) * U)
            for j0 in range(0, NB, 2):
                def f(b=b, usl=usl, j0=j0, p=p):
                    pool_ = psscr if (j0 // 2) % 2 == 0 else psb
                    tag_ = "scr" if (j0 // 2) % 2 == 0 else "pb"
                    pyc = pool_.tile([NBW, 2 * 3 * U], F32, name="pyc",
                                     tag=tag_)
                    for i in range(2):
                        nsl = slice((j0 + i) * NBW, (j0 + i + 1) * NBW)
                        # start zeroes the whole PSUM bank region: only the
                        # first matmul of the pair may set it
                        nc.tensor.matmul(pyc[:, i * 3 * U:(i + 1) * 3 * U],
                                         xT[b][:, nsl], kcall,
                                         start=(i == 0), stop=(i == 1))
                    src = pyc.rearrange("p (j m u) -> p j m u", j=2, m=3)
                    if (b * NB + j0) % 4 < 1:
                        nc.vector.tensor_copy(yc[p][:, j0:j0 + 2, :, usl], src)
                    else:
                        nc.scalar.activation(yc[p][:, j0:j0 + 2, :, usl], src,
                                             AF.Copy)
                th.append(f)
        return th

    def egen_thunks(t):
        sl = slice(t * NTW, (t + 1) * NTW)
        th = []
        for j in range(NB):
            def f(j=j, t=t, sl=sl):
                pe = psscr.tile([NBW, NTW], F32, name="pe", tag="scr")
                nc.tensor.matmul(pe, KT[:, :, j * NBW:(j + 1) * NBW],
                                 QT[:, :, sl], start=True, stop=True,
                                 perf_mode=DR)
                nc.scalar.activation(et[t][:, j, :], pe, AF.Exp, scale=0.125,
                                     bias=lnal)
            th.append(f)
        return th

    def agroup1_thunks(b, t, a1, a2, pa):
        sl = slice(t * NTW, (t + 1) * NTW)
        head = [lambda: nc.tensor.matmul(pa, kk0, xT[b][:, sl],
                                         start=True, stop=False)]
        tail = []
        for m, asl, out in ((0, a1, head), (1, a2, tail)):
            for jp in range(NJP):
                def f(m=m, asl=asl, jp=jp, b=b):
                    js = slice(2 * jp, 2 * jp + 2)
                    nc.tensor.matmul(pa, y[b][:, js, m, :], asl[:, js, :],
                                     start=False,
                                     stop=(m == 1 and jp == NJP - 1),
                                     perf_mode=DR)
                out.append(f)
        return head, tail

    def dblock(t):
        sl = slice(t * NTW, (t + 1) * NTW)
        pd = psb.tile([1, NTW], F32, name="pd", tag="pb")
        for jp in range(NJP):
            js = slice(2 * jp, 2 * jp + 2)
            nc.tensor.matmul(pd, ones2[:, :, 0:1], et[t][:, js, :],
                             start=(jp == 0), stop=(jp == NJP - 1),
                             perf_mode=DR)
        dsb = stage.tile([1, NTW], F32, name="dsb", tag="dsb", bufs=1)
        nc.vector.tensor_add(dsb, pd, s_row[:, sl])
        rds = stage.tile([1, NTW], BF16, name="rds", tag="rds", bufs=1)
        with nc.allow_low_precision(reason="rdbc is bf16 by design"):
            nc.vector.reciprocal(rds, dsb)
        pr = psb.tile([NBW, NTW], F32, name="pr", tag="pb")
        nc.tensor.matmul(pr, ones_row, rds, start=True, stop=True)
        nc.vector.tensor_copy(rdbc[t], pr)

    def bgroup1_all(t, pas):
        """B-matmuls then a breadth-first epilogue: each DVE stage for all
        four batches before the next, so the strict-FIFO DVE queue never
        head-blocks on an ACT result while later batches' work is ready.
        pb ring is 2 deep: tmp(b) is emitted right when pb(b+2) needs the
        slot back."""
        sl = slice(t * NTW, (t + 1) * NTW)
        pbs, tmps, ssums, ths = [], [], [], []
        for b in range(BL):
            pb = psb.tile([NBW, NTW], F32, name="pb", tag="pb")
            for jp in range(NJP):
                js = slice(2 * jp, 2 * jp + 2)
                nc.tensor.matmul(pb, y[b][:, js, 2, :], et[t][:, js, :],
                                 start=(jp == 0), stop=(jp == NJP - 1),
                                 perf_mode=DR)
            pbs.append(pb)
            tmp = stage.tile([NBW, NTW], BF16, name="tmp", tag="tmp",
                             bufs=4)
            nc.vector.tensor_mul(tmp, pb, rdbc[t])
            tmps.append(tmp)
        for b in range(BL):
            ssum = stage.tile([NBW, NTW], BF16, name="ssum", tag="ssum",
                              bufs=4)
            nc.vector.tensor_add(ssum, pas[b], tmps[b])
            ssums.append(ssum)
        for b in range(BL):
            # th = tanh(GS2*ssum + bru/2): rows 0:64 -> (1+th)*h into xT
            # h-rows (factor 2 absorbed in host-halved kcall/kc0 h-rows);
            # rows 64:128 -> ut stores tanh-form u
            th = stage.tile([NBW, NTW], BF16, name="th", tag="th", bufs=4)
            nc.scalar.activation(th, ssums[b], AF.Tanh, scale=GS2,
                                 bias=bru2)
            ths.append(th)
        for b in range(BL):
            nc.vector.scalar_tensor_tensor(
                xT[b][0:U, sl], ths[b][0:U, :], 1.0, xT[b][0:U, sl],
                op0=OP.add, op1=OP.mult)
        for b in range(BL):
            p, half = b // 2, b % 2
            nc.scalar.activation(ut[p][half * U:(half + 1) * U, sl],
                                 ths[b][U:2 * U, :], AF.Copy)

    # ==================== pre-phase ====================
    pre = prelude_thunks() + ygen_thunks(0) + ygen_thunks(1)
    _interleave(pre, egen_thunks(0), ratio=6)

    # ==================== phase 1: r & u gates ====================
    for t in range(NT):
        sl = slice(t * NTW, (t + 1) * NTW)
        a1, a2 = a1c[t], a2c[t]

        pas = []
        heads, tails = [], []
        for b in range(BL):
            pa = psacc.tile([NBW, NTW], F32, name="pa", tag="acc")
            hd, tl = agroup1_thunks(b, t, a1, a2, pa)
            heads.extend(hd)
            tails.extend(tl)
            pas.append(pa)
            if t == 0 and b == 0:
                heads.extend(ygen_thunks(2))
            if t == 0 and b == 1:
                heads.extend(ygen_thunks(3))
        # id + a1 contributions for every batch first: at t=0 the a2 slice
        # is still streaming in while these run
        _interleave(heads + tails,
                    egen_thunks(t + 1) if t + 1 < NT else [], ratio=4)

        if t + 1 < NT:
            nc.sync.dma_start(out=a1c[t + 1], in_=d["a1q"][t + 1])
            nc.sync.dma_start(out=a2c[t + 1], in_=d["a2q"][t + 1])

        if t == 0:
            dblock(0)
        bgroup1_all(t, pas)
        # next tile's normalizer now: its exps completed during this tile's
        # A-groups, and emitting it here keeps rdbc(t+1) clear of the
        # B-epilogue critical path
        if t + 1 < NT:
            dblock(t + 1)

    # ==================== phase 2: c gate & h_new ====================
    # node-blocks j<12 depend only on t0..t2 epilogues — emit them first so
    # the PE keeps busy while the t3 epilogue chain drains; yc[1]'s late
    # blocks are interleaved into phase-2 t0's first A-group
    yc0, yc1 = ycgen_thunks(0), ycgen_thunks(1)
    for f in [f for i, f in enumerate(yc0) if (i % 8) < 6]:
        f()
    for f in [f for i, f in enumerate(yc0) if (i % 8) >= 6]:
        f()
    for f in [f for i, f in enumerate(yc1) if (i % 8) < 6]:
        f()
    yc1_rest = [f for i, f in enumerate(yc1) if (i % 8) >= 6]

    for t in range(NT):
        sl = slice(t * NTW, (t + 1) * NTW)
        a1, a2 = a1c[t], a2c[t]

        if t == 0:
            hps = []
            for p in range(BL // 2):
                hp = p3p.tile([NBW, NTW], BF16, name="hp", tag="hp", bufs=3)
                nc.sync.dma_start(out=hp,
                                  in_=d["hT"][2 * p:2 * p + 2, :, sl])
                hps.append(hp)

        pas = []
        for p in range(BL // 2):
            b0, b1 = 2 * p, 2 * p + 1
            pa = psacc.tile([NBW, NTW], F32, name="pa2", tag="acc")
            # first matmul of the group must span all 128 partitions with
            # start=True (start zeroes the whole bank region); the two
            # half-partition identity matmuls then accumulate.
            amain = [lambda: nc.tensor.matmul(
                         pa, yc[p][:, 0:2, 0, :], a1[:, 0:2, :],
                         start=True, stop=False, perf_mode=DR),
                     lambda: nc.tensor.matmul(
                         pa[0:U, :], kc0, xT[b0][:, sl],
                         start=False, stop=False),
                     lambda: nc.tensor.matmul(
                         pa[U:2 * U, :], kc0, xT[b1][:, sl],
                         start=False, stop=False)]
            for m, asl in ((0, a1), (1, a2)):
                for jp in range(NJP):
                    if m == 0 and jp == 0:
                        continue
                    def f(m=m, asl=asl, jp=jp, p=p, pa=pa):
                        js = slice(2 * jp, 2 * jp + 2)
                        nc.tensor.matmul(pa, yc[p][:, js, m, :],
                                         asl[:, js, :], start=False,
                                         stop=(m == 1 and jp == NJP - 1),
                                         perf_mode=DR)
                    amain.append(f)
            extra = yc1_rest if (t == 0 and p == 0) else []
            _interleave(amain, extra, ratio=3)
            pas.append(pa)

        # prefetch next tile's h while this tile's B-groups run
        if t + 1 < NT:
            nhps = []
            nsl = slice((t + 1) * NTW, (t + 2) * NTW)
            for p in range(BL // 2):
                hp = p3p.tile([NBW, NTW], BF16, name="hp", tag="hp", bufs=3)
                nc.sync.dma_start(out=hp,
                                  in_=d["hT"][2 * p:2 * p + 2, :, nsl])
                nhps.append(hp)

        pbs, tmps, ssums, cts = [], [], [], []
        for p in range(BL // 2):
            pb = psb.tile([NBW, NTW], F32, name="pb2", tag="pb")
            for jp in range(NJP):
                js = slice(2 * jp, 2 * jp + 2)
                nc.tensor.matmul(pb, yc[p][:, js, 2, :], et[t][:, js, :],
                                 start=(jp == 0), stop=(jp == NJP - 1),
                                 perf_mode=DR)
            pbs.append(pb)
            tmp = stage.tile([NBW, NTW], BF16, name="tmp2", tag="tmp",
                             bufs=4)
            nc.vector.tensor_mul(tmp, pb, rdbc[t])
            tmps.append(tmp)
        for p in range(BL // 2):
            ssum = stage.tile([NBW, NTW], BF16, name="ssum2", tag="ssum",
                              bufs=4)
            nc.vector.tensor_add(ssum, pas[p], tmps[p])
            ssums.append(ssum)
        for p in range(BL // 2):
            # out holds 2*h_new = (h-c)*tu + (h+c); the host halves it.
            # plain tensor_tensor ops get the DVE 2x 16-bit mode, unlike
            # scalar_tensor_tensor which runs at 1x.
            ct = stage.tile([NBW, NTW], BF16, name="ct", tag="ct")
            nc.scalar.activation(ct, ssums[p], AF.Tanh, scale=GSCALE,
                                 bias=bc2)
            cts.append(ct)
        d2s, s2s = [], []
        for p in range(BL // 2):
            d2 = stage.tile([NBW, NTW], BF16, name="d2", tag="d2")
            nc.vector.tensor_sub(d2, hps[p], cts[p])
            d2s.append(d2)
            s2 = stage.tile([NBW, NTW], BF16, name="s2", tag="s2")
            nc.gpsimd.tensor_add(s2, hps[p], cts[p])
            s2s.append(s2)
        for p in range(BL // 2):
            nc.vector.tensor_mul(d2s[p], d2s[p], ut[p][:, sl])
            t1 = p3p.tile([NBW, NTW], BF16, name="t1", tag="t1")
            nc.vector.tensor_add(t1, d2s[p], s2s[p])
            nc.sync.dma_start(out=out_h[2 * p:2 * p + 2, :, sl], in_=t1)
        if t + 1 < NT:
            hps = nhps

    ctx.close()


_CACHE = {}


def _get_program():
    if "nc" not in _CACHE:
        _CACHE["nc"] = _build_program()
    return _CACHE["nc"]


def _prep_inputs(inputs, h_prev, adj1, adj2, feat, SE, Wq, Wk, Ws1, bs1, Ws2,
                 bs2, r_kernel, r_bias, u_kernel, u_bias, c_kernel, c_bias):
    bf = ml_dtypes.bfloat16
    f8 = ml_dtypes.float8_e4m3
    f32 = np.float32
    perm = list(range(DIN, FROWS)) + list(range(DIN))  # [h(64); inputs(2)]

    h3 = np.asarray(h_prev, f32).reshape(B, N, U)
    hT = np.ascontiguousarray(h3.transpose(0, 2, 1))            # [B, U, N]
    inT = np.asarray(inputs, f32).transpose(0, 2, 1)            # [B, DIN, N]
    xT = np.concatenate([hT, inT], axis=1).astype(bf)           # [B, 66, N]
    hTb = hT.astype(bf)

    rk = np.asarray(r_kernel, f32)[:, perm, :]
    uk = np.asarray(u_kernel, f32)[:, perm, :]
    ck = np.asarray(c_kernel, f32)[:, perm, :]
    kkall = (BETA * np.concatenate(
        [np.concatenate([rk[m], uk[m]], axis=1) for m in (1, 2, 3)],
        axis=1)).astype(bf)                                     # [66, 384]
    kk0 = (ALPHA * BETA * np.concatenate([rk[0], uk[0]], axis=1)).astype(bf)
    # xT h-rows hold (1+tanh)*h = 2*r*h after phase 1; halve the c-kernel
    # h-rows to compensate
    ck = ck.copy()
    ck[:, 0:U, :] *= 0.5
    kcall = (BETA * np.concatenate([ck[1], ck[2], ck[3]], axis=1)).astype(bf)
    kc0 = (ALPHA * BETA * ck[0]).astype(bf)

    def adj_tiles(a):
        aT = np.asarray(a, f32).T * ALPHA
        np.clip(aT, -240.0, 240.0, out=aT)
        q = aT.astype(f8)                                      # [n, m] = A^T
        # tile[t, p, j, w] = A^T[j*128 + p, t*512 + w]
        return np.ascontiguousarray(
            q.reshape(NB, NBW, NT, NTW).transpose(2, 1, 0, 3))

    shared = {
        "a1q": adj_tiles(adj1),
        "a2q": adj_tiles(adj2),
        "fsT": np.ascontiguousarray(
            np.concatenate([np.asarray(feat, f32).T, np.asarray(SE, f32).T],
                           axis=0)).astype(bf),
        "wq": np.asarray(Wq, f32).astype(bf),
        "wk": np.asarray(Wk, f32).astype(bf),
        "ws1": np.asarray(Ws1, f32).astype(bf),
        "bs1v": np.asarray(bs1, f32).reshape(U, 1),
        "ws2a": (ALPHA * np.asarray(Ws2, f32)).reshape(U, 1).astype(bf),
        "bs2a": (ALPHA * np.asarray(bs2, f32)).reshape(1, 1),
        "kkall": kkall,
        "kk0": kk0,
        "kcall": kcall,
        "kc0": kc0,
        "bru2": 0.5 * np.concatenate(
            [np.asarray(r_bias, f32).mean(0),
             np.asarray(u_bias, f32).mean(0)]).reshape(-1, 1),
        "bc2": np.tile(np.asarray(c_bias, f32).mean(0), 2).reshape(-1, 1),
    }
    in_maps = []
    for c in range(NCORES):
        bsl = slice(c * BL, (c + 1) * BL)
        m = dict(shared)
        m["xT"] = np.ascontiguousarray(xT[bsl])
        m["hT"] = np.ascontiguousarray(hTb[bsl])
        in_maps.append(m)
    return in_maps


def kernel(**inputs):
    os.environ.setdefault("NEURON_RT_RESET_CORES", "1")
    nc = _get_program()
    in_maps = _prep_inputs(**inputs)
    res = None
    err = None
    for _ in range(2):
        try:
            res = run_bass_kernel_spmd(nc, in_maps, list(range(NCORES)))
            break
        except Exception as e:  # e.g. a wedged device; retry once
            err = e
    if res is None:
        raise err
    outs = []
    for c in range(NCORES):
        o = np.asarray(res.results[c]["out"], dtype=np.float32)
        outs.append(0.5 * o.transpose(0, 2, 1).reshape(BL, N * U))
    return np.concatenate(outs, axis=0).astype(np.float32)
